# revision 4
# baseline (speedup 1.0000x reference)
"""Trainium2 Bass kernel for nn_ArchDecoder: two stacked LSTMs (H=2048, H=4096)
unrolled DEPTH=12 sequential steps, batch=1, tensor-parallel across 8 NeuronCores.

v2 over the previous baseline:
- The per-step merged AllGather is split in two: gather_B carries
  [c_hp | h_hp | hp-logit-partials] and launches right after the B-side gate
  math; gather_A carries [h_a | arch-logit-partials] of the NEXT step and
  launches ~5us later. The A-side matmuls execute during gather_B's flight, so
  the collective round-trip is (mostly) off the critical path.
- All Sigmoids are computed as 0.5*(1+tanh(x/2)) via ACT(Tanh, scale=0.5) so
  the only ScalarE table set needed is exp_and_others (exp+tanh+relu): zero
  ACT_TABLE_LOADs in steady state (the baseline paid 2x 1.28us per step).
  States are kept in "2x space" (c_hat=2c, h_hat=2h) with the compensating 0.5
  folded into W_hh_a/W_hh_hp/W_sum/W_out_* host-side.
- Weight DMAs are reordered (init + A-side weights first, the 16.8MB W_hh_hp
  image last) so the first matmuls start ~60us earlier.
Gates are permuted host-side to [i, f, o, g] so the three tanh(x/2) groups are
contiguous (one ACT call) and g gets its own.
"""
import sys

for _p in ("/opt/trn_rl_repo", "/root/.axon_site", "/root/.axon_site/_ro/pypackages"):
    if _p not in sys.path:
        sys.path.insert(0, _p)

import numpy as np
import ml_dtypes

import concourse.bass as bass
import concourse.bacc as bacc
import concourse.mybir as mybir
import concourse.tile as tile
from concourse import bass_isa
from concourse.bass_utils import run_bass_kernel_spmd

NC = 8
V = 256
HA = 2048
HHP = 4096
DEPTH = 12
BF = mybir.dt.bfloat16
F32 = mybir.dt.float32
AF = mybir.ActivationFunctionType

SA = HA // NC          # 256 h_a positions per core
SS = HA // NC          # 256 h_sum positions per core
SHP = SA + SS          # 512 hp-state positions per core
MA = 4 * SA // 128     # 8  M-tiles for arch gates
MHP = 4 * SHP // 128   # 16 M-tiles for hp gates
KA = (V + HA) // 128   # 18 K-chunks for arch gates ([a ; h_a])
KHP_C = HHP // 128     # 32 c_hp K-chunks
KHP_I = (2 * V) // 128 # 4 inp K-chunks
KSUM = HHP // 128      # 32
CB = 10                # gather_B payload cols: [c(4) | h(4) | lpB(2)]
CA = 4                 # gather_A payload cols: [h_a(2) | lpA(2)]


def _build_nc():
    nc = bacc.Bacc(None, target_bir_lowering=False, num_devices=NC)

    wa_e = nc.declare_dram_parameter("wa", [128, MA * KA * 128], BF, isOutput=False)
    wsum_e = nc.declare_dram_parameter("wsum", [128, 2 * KSUM * 128], BF, isOutput=False)
    whpc_e = nc.declare_dram_parameter("whpc", [128, MHP * KHP_C * 128], BF, isOutput=False)
    whpi_e = nc.declare_dram_parameter("whpi", [128, MHP * KHP_I * 128], BF, isOutput=False)
    woa_e = nc.declare_dram_parameter("woa", [128, 2 * 2 * 128], BF, isOutput=False)
    wohp_e = nc.declare_dram_parameter("wohp", [128, 2 * 4 * 128], BF, isOutput=False)
    ba_e = nc.declare_dram_parameter("ba", [128, MA], F32, isOutput=False)
    bsum2_e = nc.declare_dram_parameter("bsum2", [128, 2], F32, isOutput=False)
    bhp_e = nc.declare_dram_parameter("bhp", [128, MHP], F32, isOutput=False)
    boa8_e = nc.declare_dram_parameter("boa8", [128, 2], F32, isOutput=False)
    bohp8_e = nc.declare_dram_parameter("bohp8", [128, 2], F32, isOutput=False)
    initA_e = nc.declare_dram_parameter("initA", [128, NC * CA], BF, isOutput=False)
    initB_e = nc.declare_dram_parameter("initB", [128, NC * CB], BF, isOutput=False)
    out_e = nc.declare_dram_parameter("out", [2, DEPTH, V], F32, isOutput=True)

    with tile.TileContext(nc, num_cores=NC) as tc:
        with (
            tc.tile_pool(name="wpool", bufs=1) as wpool,
            tc.tile_pool(name="cpool", bufs=1) as cpool,
            tc.tile_pool(name="spool", bufs=3) as spool,
            tc.tile_pool(name="xpool", bufs=3) as xpool,
            tc.tile_pool(name="psA", bufs=2, space="PSUM") as psA,
            tc.tile_pool(name="psHP", bufs=2, space="PSUM") as psHP,
            tc.tile_pool(name="psHP2", bufs=2, space="PSUM") as psHP2,
            tc.tile_pool(name="psM", bufs=2, space="PSUM") as psM,
            tc.tile_pool(name="dram", bufs=2, space="DRAM") as dram,
        ):
            # --- weight/const tiles; DMA emission order == load order ---
            initA0 = xpool.tile([128, NC * CA], BF, tag="allA", bufs=3)
            initB0 = xpool.tile([128, NC * CB], BF, tag="allB", bufs=3)
            ba = cpool.tile([128, MA], F32, tag="ba")
            wa = wpool.tile([128, MA * KA * 128], BF, tag="wa")
            bsum2 = cpool.tile([128, 2], F32, tag="bsum2")
            wsum = wpool.tile([128, 2 * KSUM * 128], BF, tag="wsum")
            whpi = wpool.tile([128, MHP * KHP_I * 128], BF, tag="whpi")
            bhp = cpool.tile([128, MHP], F32, tag="bhp")
            woa = wpool.tile([128, 2 * 2 * 128], BF, tag="woa")
            wohp = wpool.tile([128, 2 * 4 * 128], BF, tag="wohp")
            boa8 = cpool.tile([128, 2], F32, tag="boa8")
            bohp8 = cpool.tile([128, 2], F32, tag="bohp8")
            whpc = wpool.tile([128, MHP * KHP_C * 128], BF, tag="whpc")
            nc.sync.dma_start(initA0[:], initA_e[:])
            nc.sync.dma_start(initB0[:], initB_e[:])
            nc.sync.dma_start(ba[:], ba_e[:])
            nc.sync.dma_start(wa[:], wa_e[:])
            nc.sync.dma_start(bsum2[:], bsum2_e[:])
            nc.sync.dma_start(wsum[:], wsum_e[:])
            nc.sync.dma_start(whpi[:], whpi_e[:])
            nc.sync.dma_start(bhp[:], bhp_e[:])
            nc.sync.dma_start(woa[:], woa_e[:])
            nc.sync.dma_start(wohp[:], wohp_e[:])
            nc.sync.dma_start(boa8[:], boa8_e[:])
            nc.sync.dma_start(bohp8[:], bohp8_e[:])
            nc.sync.dma_start(whpc[:], whpc_e[:])

            c2_a = cpool.tile([128, 2], F32, tag="c2_a")   # 2*c_a state
            nc.vector.memset(c2_a[:], 0.0)
            outA = cpool.tile([128, 2 * DEPTH], F32, tag="outA")
            outHP = cpool.tile([128, 2 * DEPTH], F32, tag="outHP")

            a_bf0 = xpool.tile([128, 2], BF, tag="a_bf")
            ahp_bf0 = xpool.tile([128, 2], BF, tag="ahp_bf")
            nc.vector.memset(a_bf0[:], 1.0 / V)
            nc.vector.memset(ahp_bf0[:], 1.0 / V)

            # column helpers inside gathered [128, NC*C] tiles (rank-major)
            ccol = lambda kc: (kc // 4) * CB + (kc % 4)           # c_hat chunks
            hcol = lambda kc: (kc // 4) * CB + 4 + (kc % 4)       # h_hat chunks
            acol = lambda j: (j // 2) * CA + (j % 2)              # h_a_hat chunks

            def A_step(a_bf, allA):
                """arch LSTM step: gates, state update; returns (ph2_next, payA).
                ph2_next[:,0:2] = h_a_hat slice; payA = [h_a_hat bf | lpA]."""
                ga_ps = psA.tile([128, MA], F32, tag="ga_ps")
                for m in range(MA):
                    for kc in list(range(2, KA)) + [0, 1]:
                        if kc < 2:
                            rhs = a_bf[:, kc:kc + 1]
                        else:
                            c = acol(kc - 2)
                            rhs = allA[:, c:c + 1]
                        nc.tensor.matmul(
                            ga_ps[:, m:m + 1],
                            wa[:, (m * KA + kc) * 128:(m * KA + kc + 1) * 128],
                            rhs, start=(kc == 2), stop=(kc == 1),
                        )
                ga_sb = spool.tile([128, MA], F32, tag="ga_sb")
                nc.vector.tensor_add(ga_sb[:], ga_ps[:], ba[:])
                acts = spool.tile([128, MA], F32, tag="acts_a")
                # gate layout [i(0:2) f(2:4) o(4:6) g(6:8)]
                nc.scalar.activation(acts[:, 0:6], ga_sb[:, 0:6], AF.Tanh, scale=0.5)
                nc.scalar.activation(acts[:, 6:8], ga_sb[:, 6:8], AF.Tanh)
                m1 = spool.tile([128, 2], F32, tag="am1")
                s1 = spool.tile([128, 2], F32, tag="as1")
                nc.vector.tensor_mul(m1[:], acts[:, 0:2], acts[:, 6:8])
                nc.vector.tensor_add(s1[:], acts[:, 6:8], m1[:])
                m2 = spool.tile([128, 2], F32, tag="am2")
                s2 = spool.tile([128, 2], F32, tag="as2")
                nc.vector.tensor_mul(m2[:], acts[:, 2:4], c2_a[:])
                nc.vector.tensor_add(s2[:], c2_a[:], m2[:])
                s2h = spool.tile([128, 2], F32, tag="as2h")
                nc.vector.tensor_scalar_mul(s2h[:], s2[:], 0.5)
                nc.vector.tensor_add(c2_a[:], s1[:], s2h[:])
                tc_a = spool.tile([128, 2], F32, tag="tc_a")
                nc.scalar.activation(tc_a[:], c2_a[:], AF.Tanh, scale=0.5)
                ph2 = spool.tile([128, 4], F32, tag="ph2")
                m3 = spool.tile([128, 2], F32, tag="am3")
                nc.vector.tensor_mul(m3[:], acts[:, 4:6], tc_a[:])
                nc.vector.tensor_add(ph2[:, 0:2], tc_a[:], m3[:])
                payA = spool.tile([128, CA], BF, tag="payA")
                nc.vector.tensor_copy(payA[:, 0:2], ph2[:, 0:2])
                la_ps = psM.tile([128, 2], F32, tag="psM")
                for m in range(2):
                    for kc in range(2):
                        nc.tensor.matmul(
                            la_ps[:, m:m + 1],
                            woa[:, (m * 2 + kc) * 128:(m * 2 + kc + 1) * 128],
                            payA[:, kc:kc + 1], start=(kc == 0), stop=(kc == 1),
                        )
                nc.vector.tensor_add(payA[:, 2:4], la_ps[:], boa8[:])
                return ph2, payA

            def gather(pay, cols, tagc, tagg, taga):
                cc = dram.tile([128, cols], BF, tag=tagc)
                g = dram.tile([NC, 128, cols], BF, tag=tagg)
                nc.sync.dma_start(cc[:], pay[:])
                nc.gpsimd.collective_compute(
                    "AllGather", mybir.AluOpType.bypass,
                    replica_groups=[list(range(NC))],
                    ins=[cc.opt()], outs=[g.opt()],
                )
                return g

            def readback(g, cols, taga):
                allT = xpool.tile([128, NC * cols], BF, tag=taga)
                nc.sync.dma_start(allT[:], g[:].rearrange("r p c -> p r c"))
                return allT

            def softmax(allT, cols, lo, t, outT, tag):
                """reduce rank partials -> exp -> normalize; returns prob bf16."""
                red = spool.tile([128, 2], F32, tag=f"red{tag}")
                nc.vector.tensor_reduce(
                    red[:],
                    allT[:].rearrange("p (r c) -> p c r", r=NC)[:, lo:lo + 2, :],
                    mybir.AxisListType.X, mybir.AluOpType.add,
                )
                ex = spool.tile([128, 2], F32, tag=f"ex{tag}")
                sfree = spool.tile([128, 1], F32, tag=f"sf{tag}")
                nc.scalar.activation(ex[:], red[:], AF.Exp, accum_out=sfree[:])
                spart = spool.tile([128, 1], F32, tag=f"sp{tag}")
                nc.gpsimd.partition_all_reduce(spart[:], sfree[:], 128, bass_isa.ReduceOp.add)
                zinv = spool.tile([128, 1], F32, tag=f"zi{tag}")
                nc.vector.reciprocal(zinv[:], spart[:])
                nc.vector.tensor_scalar_mul(outT[:, 2 * t:2 * t + 2], ex[:], zinv[:, 0:1])
                prob = xpool.tile([128, 2], BF, tag=f"{tag}_bf")
                nc.vector.tensor_copy(prob[:], outT[:, 2 * t:2 * t + 2])
                return prob

            # ---------------- preamble: A-step 0 + its gather ----------------
            ph2_cur, payA0 = A_step(a_bf0, initA0)
            gA = gather(payA0, CA, "ccA", "gA", "allA")
            allA_cur = readback(gA, CA, "allA")

            allB_prev = initB0
            ahp_bf = ahp_bf0

            for t in range(DEPTH):
                # softmaxes (vector/scalar/gpsimd; hidden under the MM phase)
                if t > 0:
                    ahp_bf = softmax(allB_prev, CB, 8, t - 1, outHP, "ahp")
                a_bf = softmax(allA_cur, CA, 2, t, outA, "a")

                # --- B-side MMs ---
                hs_ps = psM.tile([128, 2], F32, tag="psM")
                for m in range(2):
                    for kc in range(KSUM):
                        nc.tensor.matmul(
                            hs_ps[:, m:m + 1],
                            wsum[:, (m * KSUM + kc) * 128:(m * KSUM + kc + 1) * 128],
                            allB_prev[:, hcol(kc):hcol(kc) + 1],
                            start=(kc == 0), stop=(kc == KSUM - 1),
                        )
                ghp_ps = None
                if t > 0:
                    ghp_ps = psHP.tile([128, MHP], F32, tag="ghp_ps")
                    for m in range(MHP):
                        for kc in range(KHP_C):
                            nc.tensor.matmul(
                                ghp_ps[:, m:m + 1],
                                whpc[:, (m * KHP_C + kc) * 128:(m * KHP_C + kc + 1) * 128],
                                allB_prev[:, ccol(kc):ccol(kc) + 1],
                                start=(kc == 0), stop=(kc == KHP_C - 1),
                            )
                # h_sum_hat = 2*relu(W_sum@h + b_sum) = relu(2*psum + 2*b_sum)
                nc.scalar.activation(ph2_cur[:, 2:3], hs_ps[:, 0:1], AF.Relu,
                                     bias=bsum2[:, 0:1], scale=2.0)
                nc.scalar.activation(ph2_cur[:, 3:4], hs_ps[:, 1:2], AF.Relu,
                                     bias=bsum2[:, 1:2], scale=2.0)
                ghp2_ps = psHP2.tile([128, MHP], F32, tag="ghp2_ps")
                for m in range(MHP):
                    for j in range(KHP_I):
                        rhs = a_bf[:, j:j + 1] if j < 2 else ahp_bf[:, j - 2:j - 1]
                        nc.tensor.matmul(
                            ghp2_ps[:, m:m + 1],
                            whpi[:, (m * KHP_I + j) * 128:(m * KHP_I + j + 1) * 128],
                            rhs, start=(j == 0), stop=(j == KHP_I - 1),
                        )

                # --- B-side gate math (DVE/ACT) ---
                ghp_sb = spool.tile([128, MHP], F32, tag="ghp_sb")
                if t > 0:
                    gsum = spool.tile([128, MHP], F32, tag="gsum")
                    nc.vector.tensor_add(gsum[:], ghp_ps[:], bhp[:])
                    nc.vector.tensor_add(ghp_sb[:], gsum[:], ghp2_ps[:])
                else:
                    nc.vector.tensor_add(ghp_sb[:], ghp2_ps[:], bhp[:])
                acts_h = spool.tile([128, MHP], F32, tag="acts_h")
                # gate layout [i(0:4) f(4:8) o(8:12) g(12:16)]
                nc.scalar.activation(acts_h[:, 0:12], ghp_sb[:, 0:12], AF.Tanh, scale=0.5)
                nc.scalar.activation(acts_h[:, 12:16], ghp_sb[:, 12:16], AF.Tanh)
                hm1 = spool.tile([128, 4], F32, tag="hm1")
                hs1 = spool.tile([128, 4], F32, tag="hs1")
                nc.vector.tensor_mul(hm1[:], acts_h[:, 0:4], acts_h[:, 12:16])
                nc.vector.tensor_add(hs1[:], acts_h[:, 12:16], hm1[:])
                hm2 = spool.tile([128, 4], F32, tag="hm2")
                hs2 = spool.tile([128, 4], F32, tag="hs2")
                nc.vector.tensor_mul(hm2[:], acts_h[:, 4:8], ph2_cur[:])
                nc.vector.tensor_add(hs2[:], ph2_cur[:], hm2[:])
                hs2h = spool.tile([128, 4], F32, tag="hs2h")
                nc.vector.tensor_scalar_mul(hs2h[:], hs2[:], 0.5)
                c2hp = spool.tile([128, 4], F32, tag="c2hp")
                nc.vector.tensor_add(c2hp[:], hs1[:], hs2h[:])
                payB = spool.tile([128, CB], BF, tag="payB")
                nc.vector.tensor_copy(payB[:, 0:4], c2hp[:])
                tch = spool.tile([128, 4], F32, tag="tch")
                nc.scalar.activation(tch[:], c2hp[:], AF.Tanh, scale=0.5)
                hm3 = spool.tile([128, 4], F32, tag="hm3")
                h2 = spool.tile([128, 4], F32, tag="h2")
                nc.vector.tensor_mul(hm3[:], acts_h[:, 8:12], tch[:])
                nc.vector.tensor_add(h2[:], tch[:], hm3[:])
                nc.vector.tensor_copy(payB[:, 4:8], h2[:])

                lhp_ps = psM.tile([128, 2], F32, tag="psM")
                for m in range(2):
                    for kc in range(4):
                        nc.tensor.matmul(
                            lhp_ps[:, m:m + 1],
                            wohp[:, (m * 4 + kc) * 128:(m * 4 + kc + 1) * 128],
                            payB[:, 4 + kc:5 + kc], start=(kc == 0), stop=(kc == 3),
                        )
                nc.vector.tensor_add(payB[:, 8:10], lhp_ps[:], bohp8[:])

                gB = gather(payB, CB, "ccB", "gB", "allB")

                if t + 1 < DEPTH:
                    ph2_next, payA = A_step(a_bf, allA_cur)
                    gA = gather(payA, CA, "ccA", "gA", "allA")

                allB_prev = readback(gB, CB, "allB")
                if t + 1 < DEPTH:
                    allA_cur = readback(gA, CA, "allA")
                    ph2_cur = ph2_next

            # tail: last hp softmax
            softmax(allB_prev, CB, 8, DEPTH - 1, outHP, "ahp")

            for t in range(DEPTH):
                nc.sync.dma_start(
                    out_e[0, t].rearrange("(m p) -> p m", p=128),
                    outA[:, 2 * t:2 * t + 2],
                )
                nc.sync.dma_start(
                    out_e[1, t].rearrange("(m p) -> p m", p=128),
                    outHP[:, 2 * t:2 * t + 2],
                )
    nc.finalize()
    return nc


_NC_CACHE = None


def _get_nc():
    global _NC_CACHE
    if _NC_CACHE is None:
        _NC_CACHE = _build_nc()
    return _NC_CACHE


def _lhsT_pack(w_cat, n_m, n_k):
    """w_cat [n_m*128 rows, n_k*128 cols] -> SBUF image [128, n_m*n_k*128] where
    cols [(m*n_k+kc)*128 + j] on partition p = w_cat[m*128 + j, kc*128 + p]."""
    a = w_cat.reshape(n_m, 128, n_k, 128)           # [m, j, kc, p]
    return np.ascontiguousarray(a.transpose(3, 0, 2, 1).reshape(128, n_m * n_k * 128))


GATE_PERM = (0, 1, 3, 2)  # pytorch [i,f,g,o] -> kernel [i,f,o,g]


def _prep_in_maps(x_thought_vec_arch, x_thought_vec_arch_hp,
                  W_ih_a, W_hh_a, b_ih_a, b_hh_a, W_out_a, b_out_a,
                  W_sum, b_sum, W_ih_hp, W_hh_hp, b_ih_hp, b_hh_hp,
                  W_out_hp, b_out_hp):
    f32 = np.float32
    bf16 = ml_dtypes.bfloat16
    php = np.concatenate([
        np.concatenate([np.arange(SA * k, SA * (k + 1)),
                        HA + np.arange(SS * k, SS * (k + 1))])
        for k in range(NC)
    ])
    ba_full = (np.asarray(b_ih_a) + np.asarray(b_hh_a)).astype(f32)
    bhp_full = (np.asarray(b_ih_hp) + np.asarray(b_hh_hp)).astype(f32)
    ha0 = np.asarray(x_thought_vec_arch, f32).reshape(HA)
    hhp0 = np.asarray(x_thought_vec_arch_hp, f32).reshape(HHP)
    W_ih_a = np.asarray(W_ih_a, f32); W_hh_a = np.asarray(W_hh_a, f32)
    W_out_a = np.asarray(W_out_a, f32); W_sum = np.asarray(W_sum, f32)
    W_ih_hp = np.asarray(W_ih_hp, f32); W_hh_hp = np.asarray(W_hh_hp, f32)
    W_out_hp = np.asarray(W_out_hp, f32)
    b_out_a = np.asarray(b_out_a, f32); b_out_hp = np.asarray(b_out_hp, f32)
    b_sum = np.asarray(b_sum, f32)

    # init images in "2x" space, laid out like the gathered tiles
    initA = np.zeros((128, NC * CA), f32)
    initB = np.zeros((128, NC * CB), f32)
    ha0_2 = 2.0 * ha0
    hhp0_2 = (2.0 * hhp0)[php]
    for r in range(NC):
        for j in range(2):
            initA[:, r * CA + j] = ha0_2[r * SA + j * 128: r * SA + (j + 1) * 128]
        for q in range(4):
            initB[:, r * CB + 4 + q] = hhp0_2[r * SHP + q * 128: r * SHP + (q + 1) * 128]

    in_maps = []
    for k in range(NC):
        ja = np.arange(SA * k, SA * (k + 1))
        rows_a = np.concatenate([g * HA + ja for g in GATE_PERM])
        wa_cat = np.concatenate([W_ih_a[rows_a], 0.5 * W_hh_a[rows_a]], axis=1)
        jhp = php[SHP * k: SHP * (k + 1)]
        rows_hp = np.concatenate([g * HHP + jhp for g in GATE_PERM])
        whpc_cat = 0.5 * W_hh_hp[rows_hp][:, php]
        whpi_cat = W_ih_hp[rows_hp]
        js = np.arange(SS * k, SS * (k + 1))
        wsum_p = 0.5 * W_sum[js][:, php]
        woa_p = 0.5 * W_out_a[:, ja]
        wohp_p = 0.5 * W_out_hp[:, jhp]
        in_maps.append({
            "wa": _lhsT_pack(wa_cat, MA, KA).astype(bf16),
            "wsum": _lhsT_pack(wsum_p, 2, KSUM).astype(bf16),
            "whpc": _lhsT_pack(whpc_cat, MHP, KHP_C).astype(bf16),
            "whpi": _lhsT_pack(whpi_cat, MHP, KHP_I).astype(bf16),
            "woa": _lhsT_pack(woa_p, 2, 2).astype(bf16),
            "wohp": _lhsT_pack(wohp_p, 2, 4).astype(bf16),
            "ba": np.ascontiguousarray(ba_full[rows_a].reshape(MA, 128).T),
            "bsum2": np.ascontiguousarray((2.0 * b_sum[js]).reshape(2, 128).T),
            "bhp": np.ascontiguousarray(bhp_full[rows_hp].reshape(MHP, 128).T),
            "boa8": np.ascontiguousarray((b_out_a / NC).reshape(2, 128).T),
            "bohp8": np.ascontiguousarray((b_out_hp / NC).reshape(2, 128).T),
            "initA": initA.astype(bf16),
            "initB": initB.astype(bf16),
        })
    return in_maps


def _run(in_maps, trace=False):
    nc = _get_nc()
    return run_bass_kernel_spmd(nc, in_maps, core_ids=list(range(NC)), trace=trace)


def kernel(**inputs):
    in_maps = _prep_in_maps(**{k: np.asarray(v) for k, v in inputs.items()})
    res = _run(in_maps, trace=False)
    out = np.asarray(res.results[0]["out"], np.float32)
    return out[0][None], out[1][None]


def kernel_traced(**inputs):
    """Like kernel() but with NTFF profiling; returns ((arch, arch_hp), exec_time_ns)."""
    try:
        import ntff_hook
        ntff_hook.install()
    except Exception:
        pass
    in_maps = _prep_in_maps(**{k: np.asarray(v) for k, v in inputs.items()})
    res = _run(in_maps, trace=True)
    out = np.asarray(res.results[0]["out"], np.float32)
    return (out[0][None], out[1][None]), res.exec_time_ns


# revision 11
# speedup vs baseline: 1.4314x; 1.4314x over previous
"""Trainium2 Bass kernel for nn_ArchDecoder: two stacked LSTMs (H=2048, H=4096)
unrolled DEPTH=12 sequential steps, batch=1, tensor-parallel across 8 NeuronCores.

v2 over the previous baseline:
- The per-step merged AllGather is split in two: gather_B carries
  [c_hp | h_hp | hp-logit-partials] and launches right after the B-side gate
  math; gather_A carries [h_a | arch-logit-partials] of the NEXT step and
  launches ~5us later. The A-side matmuls execute during gather_B's flight, so
  the collective round-trip is (mostly) off the critical path.
- All Sigmoids are computed as 0.5*(1+tanh(x/2)) via ACT(Tanh, scale=0.5) so
  the only ScalarE table set needed is exp_and_others (exp+tanh+relu): zero
  ACT_TABLE_LOADs in steady state (the baseline paid 2x 1.28us per step).
  States are kept in "2x space" (c_hat=2c, h_hat=2h) with the compensating 0.5
  folded into W_hh_a/W_hh_hp/W_sum/W_out_* host-side.
- Weight DMAs are reordered (init + A-side weights first, the 16.8MB W_hh_hp
  image last) so the first matmuls start ~60us earlier.
Gates are permuted host-side to [i, f, o, g] so the three tanh(x/2) groups are
contiguous (one ACT call) and g gets its own.
"""
import sys

for _p in ("/opt/trn_rl_repo", "/root/.axon_site", "/root/.axon_site/_ro/pypackages"):
    if _p not in sys.path:
        sys.path.insert(0, _p)

import numpy as np
import ml_dtypes

import concourse.bass as bass
import concourse.bacc as bacc
import concourse.mybir as mybir
import concourse.tile as tile
from concourse import bass_isa
from concourse.bass_utils import run_bass_kernel_spmd

NC = 8
V = 256
HA = 2048
HHP = 4096
DEPTH = 12
BF = mybir.dt.bfloat16
F32 = mybir.dt.float32
AF = mybir.ActivationFunctionType

SA = HA // NC          # 256 h_a positions per core
SS = HA // NC          # 256 h_sum positions per core
SHP = SA + SS          # 512 hp-state positions per core
MA = 4 * SA // 128     # 8  M-tiles for arch gates
MHP = 4 * SHP // 128   # 16 M-tiles for hp gates
KA = (V + HA) // 128   # 18 K-chunks for arch gates ([a ; h_a])
KHP_C = HHP // 128     # 32 c_hp K-chunks
KHP_I = (2 * V) // 128 # 4 inp K-chunks
KSUM = HHP // 128      # 32
CB = 10                # gather_B payload cols: [c(4) | h(4) | lpB(2)]
CA = 4                 # gather_A payload cols: [h_a(2) | lpA(2)]


def _build_nc():
    nc = bacc.Bacc(None, target_bir_lowering=False, num_devices=NC)

    wa_e = nc.declare_dram_parameter("wa", [128, MA * KA * 128], BF, isOutput=False)
    wsum_e = nc.declare_dram_parameter("wsum", [128, 2 * KSUM * 128], BF, isOutput=False)
    whpc_e = nc.declare_dram_parameter("whpc", [128, MHP * KHP_C * 128], BF, isOutput=False)
    whpi_e = nc.declare_dram_parameter("whpi", [128, MHP * KHP_I * 128], BF, isOutput=False)
    woa_e = nc.declare_dram_parameter("woa", [128, 2 * 2 * 128], BF, isOutput=False)
    wohp_e = nc.declare_dram_parameter("wohp", [128, 2 * 4 * 128], BF, isOutput=False)
    ba_e = nc.declare_dram_parameter("ba", [128, MA], F32, isOutput=False)
    bsum2_e = nc.declare_dram_parameter("bsum2", [128, 2], F32, isOutput=False)
    bhp_e = nc.declare_dram_parameter("bhp", [128, MHP], F32, isOutput=False)
    boa8_e = nc.declare_dram_parameter("boa8", [128, 2], F32, isOutput=False)
    bohp8_e = nc.declare_dram_parameter("bohp8", [128, 2], F32, isOutput=False)
    initA_e = nc.declare_dram_parameter("initA", [128, NC * CA], BF, isOutput=False)
    initB_e = nc.declare_dram_parameter("initB", [128, NC * CB], BF, isOutput=False)
    ident_e = nc.declare_dram_parameter("ident", [128, 128], F32, isOutput=False)
    out_e = nc.declare_dram_parameter("out", [2, DEPTH, V], F32, isOutput=True)

    with tile.TileContext(nc, num_cores=NC) as tc:
        with (
            tc.tile_pool(name="wpool", bufs=1) as wpool,
            tc.tile_pool(name="cpool", bufs=1) as cpool,
            tc.tile_pool(name="spool", bufs=3) as spool,
            tc.tile_pool(name="xpool", bufs=3) as xpool,
            tc.tile_pool(name="psA", bufs=2, space="PSUM") as psA,
            tc.tile_pool(name="psHP", bufs=1, space="PSUM") as psHP,
            tc.tile_pool(name="psHP2", bufs=1, space="PSUM") as psHP2,
            tc.tile_pool(name="psM", bufs=2, space="PSUM") as psM,
            tc.tile_pool(name="dram", bufs=2, space="DRAM") as dram,
        ):
            # --- weight/const tiles; DMA emission order == load order ---
            initA0 = xpool.tile([128, NC * CA], BF, tag="allA", bufs=3)
            initB0 = xpool.tile([128, NC * CB], BF, tag="allB", bufs=3)
            ba = cpool.tile([128, MA], F32, tag="ba")
            wa = wpool.tile([128, MA * KA * 128], BF, tag="wa")
            bsum2 = cpool.tile([128, 2], F32, tag="bsum2")
            wsum = wpool.tile([128, 2 * KSUM * 128], BF, tag="wsum")
            whpi = wpool.tile([128, MHP * KHP_I * 128], BF, tag="whpi")
            bhp = cpool.tile([128, MHP], F32, tag="bhp")
            woa = wpool.tile([128, 2 * 2 * 128], BF, tag="woa")
            wohp = wpool.tile([128, 2 * 4 * 128], BF, tag="wohp")
            boa8 = cpool.tile([128, 2], F32, tag="boa8")
            bohp8 = cpool.tile([128, 2], F32, tag="bohp8")
            whpc = wpool.tile([128, MHP * KHP_C * 128], BF, tag="whpc")
            ident = cpool.tile([128, 128], F32, tag="ident")
            # weights go through the Scalar engine's DMA ring so the per-step
            # payload/readback DMAs (on the Sync ring) aren't queued behind 26MB
            nc.scalar.dma_start(initA0[:], initA_e[:])
            nc.scalar.dma_start(initB0[:], initB_e[:])
            nc.scalar.dma_start(ba[:], ba_e[:])
            nc.scalar.dma_start(wa[:], wa_e[:])
            nc.scalar.dma_start(bsum2[:], bsum2_e[:])
            nc.scalar.dma_start(wsum[:], wsum_e[:])
            nc.scalar.dma_start(whpi[:], whpi_e[:])
            nc.scalar.dma_start(bhp[:], bhp_e[:])
            nc.scalar.dma_start(woa[:], woa_e[:])
            nc.scalar.dma_start(wohp[:], wohp_e[:])
            nc.scalar.dma_start(boa8[:], boa8_e[:])
            nc.scalar.dma_start(bohp8[:], bohp8_e[:])
            nc.scalar.dma_start(ident[:], ident_e[:])
            nc.scalar.dma_start(whpc[:], whpc_e[:])

            c2_a = cpool.tile([128, 2], F32, tag="c2_a")   # 2*c_a state
            nc.vector.memset(c2_a[:], 0.0)
            # outputs accumulate on-chip; [0:24]=arch, [24:48]=arch_hp
            outAB = cpool.tile([128, 4 * DEPTH], F32, tag="outAB")
            outA = outAB[:, 0:2 * DEPTH]
            outHP = outAB[:, 2 * DEPTH:4 * DEPTH]

            a_bf0 = xpool.tile([128, 2], BF, tag="a_bf")
            ahp_bf0 = xpool.tile([128, 2], BF, tag="ahp_bf")
            nc.vector.memset(a_bf0[:], 1.0 / V)
            nc.vector.memset(ahp_bf0[:], 1.0 / V)

            # column helpers inside gathered [128, NC*C] tiles (rank-major)
            ccol = lambda kc: (kc // 4) * CB + (kc % 4)           # c_hat chunks
            hcol = lambda kc: (kc // 4) * CB + 4 + (kc % 4)       # h_hat chunks
            acol = lambda j: (j // 2) * CA + (j % 2)              # h_a_hat chunks

            def A_step(a_bf, allA):
                """arch LSTM step: gates, state update; returns (ph2_next, payA).
                ph2_next[:,0:2] = h_a_hat slice; payA = [h_a_hat bf | lpA]."""
                ga_ps = psA.tile([128, MA], F32, tag="ga_ps")
                for m in range(MA):
                    for kc in list(range(2, KA)) + [0, 1]:
                        if kc < 2:
                            rhs = a_bf[:, kc:kc + 1]
                        else:
                            c = acol(kc - 2)
                            rhs = allA[:, c:c + 1]
                        nc.tensor.matmul(
                            ga_ps[:, m:m + 1],
                            wa[:, (m * KA + kc) * 128:(m * KA + kc + 1) * 128],
                            rhs, start=(kc == 2), stop=(kc == 1),
                        )
                ga_sb = spool.tile([128, MA], F32, tag="ga_sb")
                nc.vector.tensor_add(ga_sb[:], ga_ps[:], ba[:])
                acts = spool.tile([128, MA], F32, tag="acts_a")
                # gate layout [i(0:2) f(2:4) o(4:6) g(6:8)]
                nc.scalar.activation(acts[:, 0:6], ga_sb[:, 0:6], AF.Tanh, scale=0.5)
                nc.scalar.activation(acts[:, 6:8], ga_sb[:, 6:8], AF.Tanh)
                m1 = spool.tile([128, 2], F32, tag="am1")
                s1 = spool.tile([128, 2], F32, tag="as1")
                nc.vector.tensor_mul(m1[:], acts[:, 0:2], acts[:, 6:8])
                nc.vector.tensor_add(s1[:], acts[:, 6:8], m1[:])
                m2 = spool.tile([128, 2], F32, tag="am2")
                s2 = spool.tile([128, 2], F32, tag="as2")
                nc.vector.tensor_mul(m2[:], acts[:, 2:4], c2_a[:])
                nc.vector.tensor_add(s2[:], c2_a[:], m2[:])
                s2h = spool.tile([128, 2], F32, tag="as2h")
                nc.vector.tensor_scalar_mul(s2h[:], s2[:], 0.5)
                nc.vector.tensor_add(c2_a[:], s1[:], s2h[:])
                tc_a = spool.tile([128, 2], F32, tag="tc_a")
                nc.scalar.activation(tc_a[:], c2_a[:], AF.Tanh, scale=0.5)
                ph2 = spool.tile([128, 4], F32, tag="ph2")
                m3 = spool.tile([128, 2], F32, tag="am3")
                nc.vector.tensor_mul(m3[:], acts[:, 4:6], tc_a[:])
                nc.vector.tensor_add(ph2[:, 0:2], tc_a[:], m3[:])
                payA = spool.tile([128, CA], BF, tag="payA")
                nc.vector.tensor_copy(payA[:, 0:2], ph2[:, 0:2])
                la_ps = psM.tile([128, 2], F32, tag="psM")
                for m in range(2):
                    for kc in range(2):
                        nc.tensor.matmul(
                            la_ps[:, m:m + 1],
                            woa[:, (m * 2 + kc) * 128:(m * 2 + kc + 1) * 128],
                            payA[:, kc:kc + 1], start=(kc == 0), stop=(kc == 1),
                        )
                nc.vector.tensor_add(payA[:, 2:4], la_ps[:], boa8[:])
                return ph2, payA

            def gather(pay, cols, tagc, tagg, taga):
                cc = dram.tile([128, cols], BF, tag=tagc)
                g = dram.tile([NC, 128, cols], BF, tag=tagg)
                nc.sync.dma_start(cc[:], pay[:])
                nc.gpsimd.collective_compute(
                    "AllGather", mybir.AluOpType.bypass,
                    replica_groups=[list(range(NC))],
                    ins=[cc.opt()], outs=[g.opt()],
                )
                return g

            def readback(g, cols, taga):
                allT = xpool.tile([128, NC * cols], BF, tag=taga)
                nc.sync.dma_start(allT[:], g[:].rearrange("r p c -> p r c"))
                return allT

            def softmax(allT, cols, lo, t, outT, tag):
                """reduce rank partials -> exp -> normalize; returns prob bf16."""
                red = spool.tile([128, 2], F32, tag=f"red{tag}")
                nc.vector.tensor_reduce(
                    red[:],
                    allT[:].rearrange("p (r c) -> p c r", r=NC)[:, lo:lo + 2, :],
                    mybir.AxisListType.X, mybir.AluOpType.add,
                )
                ex = spool.tile([128, 2], F32, tag=f"ex{tag}")
                sfree = spool.tile([128, 1], F32, tag=f"sf{tag}")
                nc.scalar.activation(ex[:], red[:], AF.Exp, accum_out=sfree[:])
                spart = spool.tile([128, 1], F32, tag=f"sp{tag}")
                nc.gpsimd.partition_all_reduce(spart[:], sfree[:], 128, bass_isa.ReduceOp.add)
                zinv = spool.tile([128, 1], F32, tag=f"zi{tag}")
                nc.vector.reciprocal(zinv[:], spart[:])
                nc.vector.tensor_scalar_mul(outT[:, 2 * t:2 * t + 2], ex[:], zinv[:, 0:1])
                prob = xpool.tile([128, 2], BF, tag=f"{tag}_bf")
                nc.vector.tensor_copy(prob[:], outT[:, 2 * t:2 * t + 2])
                return prob

            # ---------------- preamble: A-step 0 + its gather ----------------
            ph2_cur, payA0 = A_step(a_bf0, initA0)
            gA = gather(payA0, CA, "ccA", "gA", "allA")
            allA_cur = readback(gA, CA, "allA")

            allB_prev = initB0
            ahp_bf = ahp_bf0

            for t in range(DEPTH):
                # softmaxes (vector/scalar/gpsimd; hidden under the MM phase)
                if t > 0:
                    ahp_bf = softmax(allB_prev, CB, 8, t - 1, outHP, "ahp")
                a_bf = softmax(allA_cur, CA, 2, t, outA, "a")

                # --- B-side MMs ---
                hs_ps = psM.tile([128, 2], F32, tag="psM")
                for m in range(2):
                    for kc in range(KSUM):
                        nc.tensor.matmul(
                            hs_ps[:, m:m + 1],
                            wsum[:, (m * KSUM + kc) * 128:(m * KSUM + kc + 1) * 128],
                            allB_prev[:, hcol(kc):hcol(kc) + 1],
                            start=(kc == 0), stop=(kc == KSUM - 1),
                        )
                ghp_ps = None
                if t > 0:
                    ghp_ps = psHP.tile([128, MHP], F32, tag="ghp_ps")
                    for m in range(MHP):
                        for kc in range(KHP_C):
                            nc.tensor.matmul(
                                ghp_ps[:, m:m + 1],
                                whpc[:, (m * KHP_C + kc) * 128:(m * KHP_C + kc + 1) * 128],
                                allB_prev[:, ccol(kc):ccol(kc) + 1],
                                start=(kc == 0), stop=(kc == KHP_C - 1),
                            )
                # h_sum_hat = 2*relu(W_sum@h + b_sum) = relu(2*psum + 2*b_sum)
                nc.scalar.activation(ph2_cur[:, 2:3], hs_ps[:, 0:1], AF.Relu,
                                     bias=bsum2[:, 0:1], scale=2.0)
                nc.scalar.activation(ph2_cur[:, 3:4], hs_ps[:, 1:2], AF.Relu,
                                     bias=bsum2[:, 1:2], scale=2.0)
                ghp2_ps = psHP2.tile([128, MHP], F32, tag="ghp2_ps")
                for m in range(MHP):
                    for j in range(KHP_I):
                        rhs = a_bf[:, j:j + 1] if j < 2 else ahp_bf[:, j - 2:j - 1]
                        nc.tensor.matmul(
                            ghp2_ps[:, m:m + 1],
                            whpi[:, (m * KHP_I + j) * 128:(m * KHP_I + j + 1) * 128],
                            rhs, start=(j == 0), stop=(j == KHP_I - 1),
                        )

                # --- B-side gate math (DVE/ACT) ---
                ghp_sb = spool.tile([128, MHP], F32, tag="ghp_sb")
                if t > 0:
                    gsum = spool.tile([128, MHP], F32, tag="gsum")
                    nc.vector.tensor_add(gsum[:], ghp_ps[:], bhp[:])
                    nc.vector.tensor_add(ghp_sb[:], gsum[:], ghp2_ps[:])
                else:
                    nc.vector.tensor_add(ghp_sb[:], ghp2_ps[:], bhp[:])
                acts_h = spool.tile([128, MHP], F32, tag="acts_h")
                # gate layout [i(0:4) f(4:8) o(8:12) g(12:16)]
                nc.scalar.activation(acts_h[:, 0:12], ghp_sb[:, 0:12], AF.Tanh, scale=0.5)
                nc.scalar.activation(acts_h[:, 12:16], ghp_sb[:, 12:16], AF.Tanh)
                hm1 = spool.tile([128, 4], F32, tag="hm1")
                hs1 = spool.tile([128, 4], F32, tag="hs1")
                nc.vector.tensor_mul(hm1[:], acts_h[:, 0:4], acts_h[:, 12:16])
                nc.vector.tensor_add(hs1[:], acts_h[:, 12:16], hm1[:])
                hm2 = spool.tile([128, 4], F32, tag="hm2")
                hs2 = spool.tile([128, 4], F32, tag="hs2")
                nc.vector.tensor_mul(hm2[:], acts_h[:, 4:8], ph2_cur[:])
                nc.vector.tensor_add(hs2[:], ph2_cur[:], hm2[:])
                hs2h = spool.tile([128, 4], F32, tag="hs2h")
                nc.vector.tensor_scalar_mul(hs2h[:], hs2[:], 0.5)
                c2hp = spool.tile([128, 4], F32, tag="c2hp")
                nc.vector.tensor_add(c2hp[:], hs1[:], hs2h[:])
                payB = spool.tile([128, CB], BF, tag="payB")
                nc.vector.tensor_copy(payB[:, 0:4], c2hp[:])
                tch = spool.tile([128, 4], F32, tag="tch")
                nc.scalar.activation(tch[:], c2hp[:], AF.Tanh, scale=0.5)
                hm3 = spool.tile([128, 4], F32, tag="hm3")
                h2 = spool.tile([128, 4], F32, tag="h2")
                nc.vector.tensor_mul(hm3[:], acts_h[:, 8:12], tch[:])
                nc.vector.tensor_add(h2[:], tch[:], hm3[:])
                nc.vector.tensor_copy(payB[:, 4:8], h2[:])

                lhp_ps = psM.tile([128, 2], F32, tag="psM")
                for m in range(2):
                    for kc in range(4):
                        nc.tensor.matmul(
                            lhp_ps[:, m:m + 1],
                            wohp[:, (m * 4 + kc) * 128:(m * 4 + kc + 1) * 128],
                            payB[:, 4 + kc:5 + kc], start=(kc == 0), stop=(kc == 3),
                        )
                nc.vector.tensor_add(payB[:, 8:10], lhp_ps[:], bohp8[:])

                gB = gather(payB, CB, "ccB", "gB", "allB")

                if t + 1 < DEPTH:
                    ph2_next, payA = A_step(a_bf, allA_cur)
                    gA = gather(payA, CA, "ccA", "gA", "allA")

                allB_prev = readback(gB, CB, "allB")
                if t + 1 < DEPTH:
                    allA_cur = readback(gA, CA, "allA")
                    ph2_cur = ph2_next

            # tail: last hp softmax
            softmax(allB_prev, CB, 8, DEPTH - 1, outHP, "ahp")

            # transpose [128, 48] -> [48, 128] on PE so the output DMA writes
            # contiguous 512B rows instead of a 4-byte-packet scatter
            tr_ps = psM.tile([4 * DEPTH, 128], F32, tag="tr_ps", bufs=1)
            nc.tensor.transpose(tr_ps[:], outAB[:], ident[:])
            trf = spool.tile([4 * DEPTH, 128], F32, tag="trf", bufs=1)
            nc.vector.tensor_copy(trf[:], tr_ps[:])
            nc.sync.dma_start(
                out_e[:].rearrange("s t (m p) -> (s t m) p", p=128),
                trf[:],
            )
    nc.finalize()
    return nc


_NC_CACHE = None


def _get_nc():
    global _NC_CACHE
    if _NC_CACHE is None:
        _NC_CACHE = _build_nc()
    return _NC_CACHE


def _lhsT_pack(w_cat, n_m, n_k):
    """w_cat [n_m*128 rows, n_k*128 cols] -> SBUF image [128, n_m*n_k*128] where
    cols [(m*n_k+kc)*128 + j] on partition p = w_cat[m*128 + j, kc*128 + p]."""
    a = w_cat.reshape(n_m, 128, n_k, 128)           # [m, j, kc, p]
    return np.ascontiguousarray(a.transpose(3, 0, 2, 1).reshape(128, n_m * n_k * 128))


GATE_PERM = (0, 1, 3, 2)  # pytorch [i,f,g,o] -> kernel [i,f,o,g]


def _prep_in_maps(x_thought_vec_arch, x_thought_vec_arch_hp,
                  W_ih_a, W_hh_a, b_ih_a, b_hh_a, W_out_a, b_out_a,
                  W_sum, b_sum, W_ih_hp, W_hh_hp, b_ih_hp, b_hh_hp,
                  W_out_hp, b_out_hp):
    f32 = np.float32
    bf16 = ml_dtypes.bfloat16
    php = np.concatenate([
        np.concatenate([np.arange(SA * k, SA * (k + 1)),
                        HA + np.arange(SS * k, SS * (k + 1))])
        for k in range(NC)
    ])
    ba_full = (np.asarray(b_ih_a) + np.asarray(b_hh_a)).astype(f32)
    bhp_full = (np.asarray(b_ih_hp) + np.asarray(b_hh_hp)).astype(f32)
    ha0 = np.asarray(x_thought_vec_arch, f32).reshape(HA)
    hhp0 = np.asarray(x_thought_vec_arch_hp, f32).reshape(HHP)
    W_ih_a = np.asarray(W_ih_a, f32); W_hh_a = np.asarray(W_hh_a, f32)
    W_out_a = np.asarray(W_out_a, f32); W_sum = np.asarray(W_sum, f32)
    W_ih_hp = np.asarray(W_ih_hp, f32); W_hh_hp = np.asarray(W_hh_hp, f32)
    W_out_hp = np.asarray(W_out_hp, f32)
    b_out_a = np.asarray(b_out_a, f32); b_out_hp = np.asarray(b_out_hp, f32)
    b_sum = np.asarray(b_sum, f32)

    # init images in "2x" space, laid out like the gathered tiles
    initA = np.zeros((128, NC * CA), f32)
    initB = np.zeros((128, NC * CB), f32)
    ha0_2 = 2.0 * ha0
    hhp0_2 = (2.0 * hhp0)[php]
    for r in range(NC):
        for j in range(2):
            initA[:, r * CA + j] = ha0_2[r * SA + j * 128: r * SA + (j + 1) * 128]
        for q in range(4):
            initB[:, r * CB + 4 + q] = hhp0_2[r * SHP + q * 128: r * SHP + (q + 1) * 128]

    in_maps = []
    for k in range(NC):
        ja = np.arange(SA * k, SA * (k + 1))
        rows_a = np.concatenate([g * HA + ja for g in GATE_PERM])
        wa_cat = np.concatenate([W_ih_a[rows_a], 0.5 * W_hh_a[rows_a]], axis=1)
        jhp = php[SHP * k: SHP * (k + 1)]
        rows_hp = np.concatenate([g * HHP + jhp for g in GATE_PERM])
        whpc_cat = 0.5 * W_hh_hp[rows_hp][:, php]
        whpi_cat = W_ih_hp[rows_hp]
        js = np.arange(SS * k, SS * (k + 1))
        wsum_p = 0.5 * W_sum[js][:, php]
        woa_p = 0.5 * W_out_a[:, ja]
        wohp_p = 0.5 * W_out_hp[:, jhp]
        in_maps.append({
            "wa": _lhsT_pack(wa_cat, MA, KA).astype(bf16),
            "wsum": _lhsT_pack(wsum_p, 2, KSUM).astype(bf16),
            "whpc": _lhsT_pack(whpc_cat, MHP, KHP_C).astype(bf16),
            "whpi": _lhsT_pack(whpi_cat, MHP, KHP_I).astype(bf16),
            "woa": _lhsT_pack(woa_p, 2, 2).astype(bf16),
            "wohp": _lhsT_pack(wohp_p, 2, 4).astype(bf16),
            "ba": np.ascontiguousarray(ba_full[rows_a].reshape(MA, 128).T),
            "bsum2": np.ascontiguousarray((2.0 * b_sum[js]).reshape(2, 128).T),
            "bhp": np.ascontiguousarray(bhp_full[rows_hp].reshape(MHP, 128).T),
            "boa8": np.ascontiguousarray((b_out_a / NC).reshape(2, 128).T),
            "bohp8": np.ascontiguousarray((b_out_hp / NC).reshape(2, 128).T),
            "initA": initA.astype(bf16),
            "initB": initB.astype(bf16),
            "ident": np.eye(128, dtype=f32),
        })
    return in_maps


def _run(in_maps, trace=False):
    nc = _get_nc()
    return run_bass_kernel_spmd(nc, in_maps, core_ids=list(range(NC)), trace=trace)


def kernel(**inputs):
    in_maps = _prep_in_maps(**{k: np.asarray(v) for k, v in inputs.items()})
    res = _run(in_maps, trace=False)
    out = np.asarray(res.results[0]["out"], np.float32)
    return out[0][None], out[1][None]


def kernel_traced(**inputs):
    """Like kernel() but with NTFF profiling; returns ((arch, arch_hp), exec_time_ns)."""
    try:
        import ntff_hook
        ntff_hook.install()
    except Exception:
        pass
    in_maps = _prep_in_maps(**{k: np.asarray(v) for k, v in inputs.items()})
    res = _run(in_maps, trace=True)
    out = np.asarray(res.results[0]["out"], np.float32)
    return (out[0][None], out[1][None]), res.exec_time_ns


# revision 12
# speedup vs baseline: 1.4981x; 1.0466x over previous
"""Trainium2 Bass kernel for nn_ArchDecoder: two stacked LSTMs (H=2048, H=4096)
unrolled DEPTH=12 sequential steps, batch=1, tensor-parallel across 8 NeuronCores.

Schedule per step (v4):
- gather_B [c_hat|h_hat|lp_hp] launches right after the B gate math; the A-side
  (next-step arch LSTM) matmuls are fenced AFTER the launch so they execute
  during the collective's flight; gather_A [h_a_hat|lp_a] trails it.
- Scheduler-only fences (tc.no_sync_barrier) pin the phase order the Tile
  scheduler would otherwise shuffle (it has no model of collective latency).
- All sigmoids are tanh(x/2)-based so the single exp_and_others ACT table set
  serves exp+tanh+relu: zero ACT_TABLE_LOADs in the loop. States are kept 2x
  scaled with the 0.5 folded into weights host-side.
- Gate biases are folded into the PSUM accumulation via one [K=16]x identity
  matmul per LSTM, so activations read raw PSUM (no DVE bias adds).
- Weights stream over the Scalar DMA ring (payloads own the Sync ring); the
  16.8MB W_hh_hp image loads last, after everything needed early.
- Outputs accumulate in SBUF and leave via one PE-transpose + single
  contiguous DMA at the end (a per-step scatter-write stalls the DMA engines).
"""
import sys

for _p in ("/opt/trn_rl_repo", "/root/.axon_site", "/root/.axon_site/_ro/pypackages"):
    if _p not in sys.path:
        sys.path.insert(0, _p)

import numpy as np
import ml_dtypes

import concourse.bass as bass
import concourse.bacc as bacc
import concourse.mybir as mybir
import concourse.tile as tile
from concourse import bass_isa
from concourse.bass_utils import run_bass_kernel_spmd

NC = 8
V = 256
HA = 2048
HHP = 4096
DEPTH = 12
BF = mybir.dt.bfloat16
F32 = mybir.dt.float32
AF = mybir.ActivationFunctionType

SA = HA // NC          # 256 h_a positions per core
SS = HA // NC          # 256 h_sum positions per core
SHP = SA + SS          # 512 hp-state positions per core
MA = 4 * SA // 128     # 8  M-tiles for arch gates
MHP = 4 * SHP // 128   # 16 M-tiles for hp gates
KA = (V + HA) // 128   # 18 K-chunks for arch gates ([a ; h_a])
KHP_C = HHP // 128     # 32 c_hp K-chunks
KHP_I = (2 * V) // 128 # 4 inp K-chunks
KSUM = HHP // 128      # 32
CB = 10                # gather_B payload cols: [c(4) | h(4) | lpB(2)]
CHL = 6                # readback cols of the h/lp part
CA = 4                 # gather_A payload cols: [h_a(2) | lpA(2)]


def _build_nc():
    nc = bacc.Bacc(None, target_bir_lowering=False, num_devices=NC)

    wa_e = nc.declare_dram_parameter("wa", [128, MA * KA * 128], BF, isOutput=False)
    wsum_e = nc.declare_dram_parameter("wsum", [128, 2 * KSUM * 128], BF, isOutput=False)
    whpc_e = nc.declare_dram_parameter("whpc", [128, MHP * KHP_C * 128], BF, isOutput=False)
    whpi_e = nc.declare_dram_parameter("whpi", [128, MHP * KHP_I * 128], BF, isOutput=False)
    woa_e = nc.declare_dram_parameter("woa", [128, 2 * 2 * 128], BF, isOutput=False)
    wohp_e = nc.declare_dram_parameter("wohp", [128, 2 * 4 * 128], BF, isOutput=False)
    baT_e = nc.declare_dram_parameter("baT", [MA, 128], BF, isOutput=False)
    bsum2_e = nc.declare_dram_parameter("bsum2", [128, 2], F32, isOutput=False)
    bhpT_e = nc.declare_dram_parameter("bhpT", [MHP, 128], BF, isOutput=False)
    identb_e = nc.declare_dram_parameter("identb", [MHP, MHP], BF, isOutput=False)
    boa8_e = nc.declare_dram_parameter("boa8", [128, 2], F32, isOutput=False)
    bohp8_e = nc.declare_dram_parameter("bohp8", [128, 2], F32, isOutput=False)
    initA_e = nc.declare_dram_parameter("initA", [128, NC * CA], BF, isOutput=False)
    initB_e = nc.declare_dram_parameter("initB", [128, NC * CHL], BF, isOutput=False)
    ident_e = nc.declare_dram_parameter("ident", [128, 128], F32, isOutput=False)
    out_e = nc.declare_dram_parameter("out", [2, DEPTH, V], F32, isOutput=True)

    with tile.TileContext(nc, num_cores=NC) as tc:
        with (
            tc.tile_pool(name="wpool", bufs=1) as wpool,
            tc.tile_pool(name="cpool", bufs=1) as cpool,
            tc.tile_pool(name="spool", bufs=3) as spool,
            tc.tile_pool(name="xpool", bufs=3) as xpool,
            tc.tile_pool(name="psA", bufs=2, space="PSUM") as psA,
            tc.tile_pool(name="psHP", bufs=1, space="PSUM") as psHP,
            tc.tile_pool(name="psM", bufs=2, space="PSUM") as psM,
            tc.tile_pool(name="dram", bufs=2, space="DRAM") as dram,
        ):
            # --- weight/const tiles; DMA emission order == load order ---
            initA0 = xpool.tile([128, NC * CA], BF, tag="allA", bufs=3)
            initB0 = xpool.tile([128, NC * CHL], BF, tag="allBhl", bufs=3)
            baT = cpool.tile([MA, 128], BF, tag="baT")
            bhpT = cpool.tile([MHP, 128], BF, tag="bhpT")
            identb = cpool.tile([MHP, MHP], BF, tag="identb")
            wa = wpool.tile([128, MA * KA * 128], BF, tag="wa")
            bsum2 = cpool.tile([128, 2], F32, tag="bsum2")
            wsum = wpool.tile([128, 2 * KSUM * 128], BF, tag="wsum")
            whpi = wpool.tile([128, MHP * KHP_I * 128], BF, tag="whpi")
            woa = wpool.tile([128, 2 * 2 * 128], BF, tag="woa")
            wohp = wpool.tile([128, 2 * 4 * 128], BF, tag="wohp")
            boa8 = cpool.tile([128, 2], F32, tag="boa8")
            bohp8 = cpool.tile([128, 2], F32, tag="bohp8")
            whpc = wpool.tile([128, MHP * KHP_C * 128], BF, tag="whpc")
            ident = cpool.tile([128, 128], F32, tag="ident")
            # weights go through the Scalar engine's DMA ring so the per-step
            # payload/readback DMAs (on the Sync ring) aren't queued behind 26MB
            nc.scalar.dma_start(initA0[:], initA_e[:])
            nc.scalar.dma_start(initB0[:], initB_e[:])
            nc.scalar.dma_start(baT[:], baT_e[:])
            nc.scalar.dma_start(bhpT[:], bhpT_e[:])
            nc.scalar.dma_start(identb[:], identb_e[:])
            nc.scalar.dma_start(wa[:], wa_e[:])
            nc.scalar.dma_start(bsum2[:], bsum2_e[:])
            nc.scalar.dma_start(wsum[:], wsum_e[:])
            nc.scalar.dma_start(whpi[:], whpi_e[:])
            nc.scalar.dma_start(woa[:], woa_e[:])
            nc.scalar.dma_start(wohp[:], wohp_e[:])
            nc.scalar.dma_start(boa8[:], boa8_e[:])
            nc.scalar.dma_start(bohp8[:], bohp8_e[:])
            nc.scalar.dma_start(ident[:], ident_e[:])
            nc.scalar.dma_start(whpc[:], whpc_e[:])

            c2_a = cpool.tile([128, 2], F32, tag="c2_a")   # 2*c_a state
            nc.vector.memset(c2_a[:], 0.0)
            # outputs accumulate on-chip; [0:24]=arch, [24:48]=arch_hp
            outAB = cpool.tile([128, 4 * DEPTH], F32, tag="outAB")
            outA = outAB[:, 0:2 * DEPTH]
            outHP = outAB[:, 2 * DEPTH:4 * DEPTH]

            a_bf0 = xpool.tile([128, 2], BF, tag="a_bf")
            ahp_bf0 = xpool.tile([128, 2], BF, tag="ahp_bf")
            nc.vector.memset(a_bf0[:], 1.0 / V)
            nc.vector.memset(ahp_bf0[:], 1.0 / V)

            # column helpers: allB_c[:, kc] is c_hat chunk kc directly;
            # h/lp live in allB_hl with rank stride CHL
            hcol = lambda kc: (kc // 4) * CHL + (kc % 4)
            acol = lambda j: (j // 2) * CA + (j % 2)

            def A_head(a_bf, allA):
                """bias matmul + first half of the arch-gate matvecs."""
                ga_ps = psA.tile([128, MA], F32, tag="ga_ps")
                nc.tensor.matmul(ga_ps[:, 0:MA], baT[:], identb[0:MA, 0:MA],
                                 start=True, stop=False)
                for m in range(MA // 2):
                    for kc in list(range(2, KA)) + [0, 1]:
                        rhs = a_bf[:, kc:kc + 1] if kc < 2 else allA[:, acol(kc - 2):acol(kc - 2) + 1]
                        nc.tensor.matmul(
                            ga_ps[:, m:m + 1],
                            wa[:, (m * KA + kc) * 128:(m * KA + kc + 1) * 128],
                            rhs, start=False, stop=(kc == 1),
                        )
                return ga_ps

            def A_rest(ga_ps, a_bf, allA):
                """second half of the gate matvecs + gate math + payA."""
                for m in range(MA // 2, MA):
                    for kc in list(range(2, KA)) + [0, 1]:
                        rhs = a_bf[:, kc:kc + 1] if kc < 2 else allA[:, acol(kc - 2):acol(kc - 2) + 1]
                        nc.tensor.matmul(
                            ga_ps[:, m:m + 1],
                            wa[:, (m * KA + kc) * 128:(m * KA + kc + 1) * 128],
                            rhs, start=False, stop=(kc == 1),
                        )
                acts = spool.tile([128, MA], F32, tag="acts_a")
                # gate layout [i(0:2) f(2:4) o(4:6) g(6:8)]
                nc.scalar.activation(acts[:, 0:6], ga_ps[:, 0:6], AF.Tanh, scale=0.5)
                nc.scalar.activation(acts[:, 6:8], ga_ps[:, 6:8], AF.Tanh)
                m1 = spool.tile([128, 2], F32, tag="am1")
                s1 = spool.tile([128, 2], F32, tag="as1")
                nc.vector.tensor_mul(m1[:], acts[:, 0:2], acts[:, 6:8])
                nc.vector.tensor_add(s1[:], acts[:, 6:8], m1[:])
                m2 = spool.tile([128, 2], F32, tag="am2")
                s2 = spool.tile([128, 2], F32, tag="as2")
                nc.vector.tensor_mul(m2[:], acts[:, 2:4], c2_a[:])
                nc.vector.tensor_add(s2[:], c2_a[:], m2[:])
                s2h = spool.tile([128, 2], F32, tag="as2h")
                nc.vector.tensor_scalar_mul(s2h[:], s2[:], 0.5)
                nc.vector.tensor_add(c2_a[:], s1[:], s2h[:])
                tc_a = spool.tile([128, 2], F32, tag="tc_a")
                nc.scalar.activation(tc_a[:], c2_a[:], AF.Tanh, scale=0.5)
                ph2 = spool.tile([128, 4], F32, tag="ph2")
                m3 = spool.tile([128, 2], F32, tag="am3")
                nc.vector.tensor_mul(m3[:], acts[:, 4:6], tc_a[:])
                nc.vector.tensor_add(ph2[:, 0:2], tc_a[:], m3[:])
                payA = spool.tile([128, CA], BF, tag="payA")
                nc.vector.tensor_copy(payA[:, 0:2], ph2[:, 0:2])
                la_ps = psM.tile([128, 2], F32, tag="psM")
                for m in range(2):
                    for kc in range(2):
                        nc.tensor.matmul(
                            la_ps[:, m:m + 1],
                            woa[:, (m * 2 + kc) * 128:(m * 2 + kc + 1) * 128],
                            payA[:, kc:kc + 1], start=(kc == 0), stop=(kc == 1),
                        )
                nc.vector.tensor_add(payA[:, 2:4], la_ps[:], boa8[:])
                return ph2, payA

            def gather(pay, cols, tagc, tagg):
                cc = dram.tile([128, cols], BF, tag=tagc)
                g = dram.tile([NC, 128, cols], BF, tag=tagg)
                nc.sync.dma_start(cc[:], pay[:])
                nc.gpsimd.collective_compute(
                    "AllGather", mybir.AluOpType.bypass,
                    replica_groups=[list(range(NC))],
                    ins=[cc.opt()], outs=[g.opt()],
                )
                return g

            def softmax(allT, lo, t, outT, tag):
                """reduce rank partials -> exp -> normalize; returns prob bf16."""
                red = spool.tile([128, 2], F32, tag=f"red{tag}")
                nc.vector.tensor_reduce(
                    red[:],
                    allT[:].rearrange("p (r c) -> p c r", r=NC)[:, lo:lo + 2, :],
                    mybir.AxisListType.X, mybir.AluOpType.add,
                )
                ex = spool.tile([128, 2], F32, tag=f"ex{tag}")
                sfree = spool.tile([128, 1], F32, tag=f"sf{tag}")
                nc.scalar.activation(ex[:], red[:], AF.Exp, accum_out=sfree[:])
                spart = spool.tile([128, 1], F32, tag=f"sp{tag}")
                nc.gpsimd.partition_all_reduce(spart[:], sfree[:], 128, bass_isa.ReduceOp.add)
                zinv = spool.tile([128, 1], F32, tag=f"zi{tag}")
                nc.vector.reciprocal(zinv[:], spart[:])
                nc.vector.tensor_scalar_mul(outT[:, 2 * t:2 * t + 2], ex[:], zinv[:, 0:1])
                prob = xpool.tile([128, 2], BF, tag=f"{tag}_bf")
                nc.vector.tensor_copy(prob[:], outT[:, 2 * t:2 * t + 2])
                return prob

            # ---------------- preamble: A-step 0 + its gather ----------------
            ga0 = A_head(a_bf0, initA0)
            ph2_cur, payA0 = A_rest(ga0, a_bf0, initA0)
            gA = gather(payA0, CA, "ccA", "gA")
            allA_cur = xpool.tile([128, NC * CA], BF, tag="allA")
            nc.sync.dma_start(allA_cur[:], gA[:].rearrange("r p c -> p r c"))

            allB_c = None
            allB_hl = initB0
            ahp_bf = ahp_bf0

            for t in range(DEPTH):
                # softmaxes first; fenced ahead of the B-side DVE work
                if t > 0:
                    ahp_bf = softmax(allB_hl, 4, t - 1, outHP, "ahp")
                a_bf = softmax(allA_cur, 2, t, outA, "a")
                tc.no_sync_barrier()

                # --- B-side MMs: one PSUM accumulation [bias | W_hh@c | W_sum?no | W_ih@inp]
                ghp_ps = psHP.tile([128, MHP], F32, tag="ghp_ps")
                nc.tensor.matmul(ghp_ps[:, 0:MHP], bhpT[:], identb[:],
                                 start=True, stop=False)
                if t > 0:
                    for m in range(MHP):
                        for kc in range(KHP_C):
                            nc.tensor.matmul(
                                ghp_ps[:, m:m + 1],
                                whpc[:, (m * KHP_C + kc) * 128:(m * KHP_C + kc + 1) * 128],
                                allB_c[:, kc:kc + 1],
                                start=False, stop=False,
                            )
                hs_ps = psM.tile([128, 2], F32, tag="psM")
                for m in range(2):
                    for kc in range(KSUM):
                        nc.tensor.matmul(
                            hs_ps[:, m:m + 1],
                            wsum[:, (m * KSUM + kc) * 128:(m * KSUM + kc + 1) * 128],
                            allB_hl[:, hcol(kc):hcol(kc) + 1],
                            start=(kc == 0), stop=(kc == KSUM - 1),
                        )
                # h_sum_hat = 2*relu(W_sum@h + b_sum) = relu(2*psum + 2*b_sum)
                nc.scalar.activation(ph2_cur[:, 2:3], hs_ps[:, 0:1], AF.Relu,
                                     bias=bsum2[:, 0:1], scale=2.0)
                nc.scalar.activation(ph2_cur[:, 3:4], hs_ps[:, 1:2], AF.Relu,
                                     bias=bsum2[:, 1:2], scale=2.0)
                for m in range(MHP):
                    for j in range(KHP_I):
                        rhs = a_bf[:, j:j + 1] if j < 2 else ahp_bf[:, j - 2:j - 1]
                        nc.tensor.matmul(
                            ghp_ps[:, m:m + 1],
                            whpi[:, (m * KHP_I + j) * 128:(m * KHP_I + j + 1) * 128],
                            rhs, start=False, stop=(j == KHP_I - 1),
                        )

                # --- B-side gate math (DVE/ACT); gates read PSUM directly ---
                acts_h = spool.tile([128, MHP], F32, tag="acts_h")
                # gate layout [i(0:4) f(4:8) o(8:12) g(12:16)]
                nc.scalar.activation(acts_h[:, 0:12], ghp_ps[:, 0:12], AF.Tanh, scale=0.5)
                nc.scalar.activation(acts_h[:, 12:16], ghp_ps[:, 12:16], AF.Tanh)
                hm1 = spool.tile([128, 4], F32, tag="hm1")
                hs1 = spool.tile([128, 4], F32, tag="hs1")
                nc.vector.tensor_mul(hm1[:], acts_h[:, 0:4], acts_h[:, 12:16])
                nc.vector.tensor_add(hs1[:], acts_h[:, 12:16], hm1[:])
                hm2 = spool.tile([128, 4], F32, tag="hm2")
                hs2 = spool.tile([128, 4], F32, tag="hs2")
                nc.vector.tensor_mul(hm2[:], acts_h[:, 4:8], ph2_cur[:])
                nc.vector.tensor_add(hs2[:], ph2_cur[:], hm2[:])
                hs2h = spool.tile([128, 4], F32, tag="hs2h")
                nc.vector.tensor_scalar_mul(hs2h[:], hs2[:], 0.5)
                payB = spool.tile([128, CB], BF, tag="payB")
                nc.vector.tensor_add(payB[:, 0:4], hs1[:], hs2h[:])   # c_hat (bf16)
                tch = spool.tile([128, 4], F32, tag="tch")
                nc.scalar.activation(tch[:], payB[:, 0:4], AF.Tanh, scale=0.5)
                hm3 = spool.tile([128, 4], F32, tag="hm3")
                nc.vector.tensor_mul(hm3[:], acts_h[:, 8:12], tch[:])
                nc.vector.tensor_add(payB[:, 4:8], tch[:], hm3[:])    # h_hat (bf16)

                # next-step A head fills the lpB-chain stall window
                ga_ps = None
                if t + 1 < DEPTH:
                    ga_ps = A_head(a_bf, allA_cur)

                lhp_ps = psM.tile([128, 2], F32, tag="psM")
                for m in range(2):
                    for kc in range(4):
                        nc.tensor.matmul(
                            lhp_ps[:, m:m + 1],
                            wohp[:, (m * 4 + kc) * 128:(m * 4 + kc + 1) * 128],
                            payB[:, 4 + kc:5 + kc], start=(kc == 0), stop=(kc == 3),
                        )
                nc.vector.tensor_add(payB[:, 8:10], lhp_ps[:], bohp8[:])

                gB = gather(payB, CB, "ccB", "gB")
                tc.no_sync_barrier()

                if t + 1 < DEPTH:
                    ph2_next, payA = A_rest(ga_ps, a_bf, allA_cur)
                    gA = gather(payA, CA, "ccA", "gA")

                allB_c = xpool.tile([128, NC * 4], BF, tag="allBc")
                nc.sync.dma_start(allB_c[:], gB[:, :, 0:4].rearrange("r p c -> p r c"))
                allB_hl = xpool.tile([128, NC * CHL], BF, tag="allBhl")
                nc.sync.dma_start(allB_hl[:], gB[:, :, 4:CB].rearrange("r p c -> p r c"))
                if t + 1 < DEPTH:
                    allA_cur = xpool.tile([128, NC * CA], BF, tag="allA")
                    nc.sync.dma_start(allA_cur[:], gA[:].rearrange("r p c -> p r c"))
                    ph2_cur = ph2_next

            # tail: last hp softmax
            softmax(allB_hl, 4, DEPTH - 1, outHP, "ahp")

            # transpose [128, 48] -> [48, 128] on PE so the output DMA writes
            # contiguous 512B rows instead of a 4-byte-packet scatter
            tr_ps = psM.tile([4 * DEPTH, 128], F32, tag="tr_ps", bufs=1)
            nc.tensor.transpose(tr_ps[:], outAB[:], ident[:])
            trf = spool.tile([4 * DEPTH, 128], F32, tag="trf", bufs=1)
            nc.vector.tensor_copy(trf[:], tr_ps[:])
            nc.sync.dma_start(
                out_e[:].rearrange("s t (m p) -> (s t m) p", p=128),
                trf[:],
            )
    nc.finalize()
    return nc


_NC_CACHE = None


def _get_nc():
    global _NC_CACHE
    if _NC_CACHE is None:
        _NC_CACHE = _build_nc()
    return _NC_CACHE


def _lhsT_pack(w_cat, n_m, n_k):
    """w_cat [n_m*128 rows, n_k*128 cols] -> SBUF image [128, n_m*n_k*128] where
    cols [(m*n_k+kc)*128 + j] on partition p = w_cat[m*128 + j, kc*128 + p]."""
    a = w_cat.reshape(n_m, 128, n_k, 128)           # [m, j, kc, p]
    return np.ascontiguousarray(a.transpose(3, 0, 2, 1).reshape(128, n_m * n_k * 128))


GATE_PERM = (0, 1, 3, 2)  # pytorch [i,f,g,o] -> kernel [i,f,o,g]


def _prep_in_maps(x_thought_vec_arch, x_thought_vec_arch_hp,
                  W_ih_a, W_hh_a, b_ih_a, b_hh_a, W_out_a, b_out_a,
                  W_sum, b_sum, W_ih_hp, W_hh_hp, b_ih_hp, b_hh_hp,
                  W_out_hp, b_out_hp):
    f32 = np.float32
    bf16 = ml_dtypes.bfloat16
    php = np.concatenate([
        np.concatenate([np.arange(SA * k, SA * (k + 1)),
                        HA + np.arange(SS * k, SS * (k + 1))])
        for k in range(NC)
    ])
    ba_full = (np.asarray(b_ih_a) + np.asarray(b_hh_a)).astype(f32)
    bhp_full = (np.asarray(b_ih_hp) + np.asarray(b_hh_hp)).astype(f32)
    ha0 = np.asarray(x_thought_vec_arch, f32).reshape(HA)
    hhp0 = np.asarray(x_thought_vec_arch_hp, f32).reshape(HHP)
    W_ih_a = np.asarray(W_ih_a, f32); W_hh_a = np.asarray(W_hh_a, f32)
    W_out_a = np.asarray(W_out_a, f32); W_sum = np.asarray(W_sum, f32)
    W_ih_hp = np.asarray(W_ih_hp, f32); W_hh_hp = np.asarray(W_hh_hp, f32)
    W_out_hp = np.asarray(W_out_hp, f32)
    b_out_a = np.asarray(b_out_a, f32); b_out_hp = np.asarray(b_out_hp, f32)
    b_sum = np.asarray(b_sum, f32)

    # init images in "2x" space, laid out like the gathered tiles
    initA = np.zeros((128, NC * CA), f32)
    initB = np.zeros((128, NC * CHL), f32)
    ha0_2 = 2.0 * ha0
    hhp0_2 = (2.0 * hhp0)[php]
    for r in range(NC):
        for j in range(2):
            initA[:, r * CA + j] = ha0_2[r * SA + j * 128: r * SA + (j + 1) * 128]
        for q in range(4):
            initB[:, r * CHL + q] = hhp0_2[r * SHP + q * 128: r * SHP + (q + 1) * 128]

    in_maps = []
    for k in range(NC):
        ja = np.arange(SA * k, SA * (k + 1))
        rows_a = np.concatenate([g * HA + ja for g in GATE_PERM])
        wa_cat = np.concatenate([W_ih_a[rows_a], 0.5 * W_hh_a[rows_a]], axis=1)
        jhp = php[SHP * k: SHP * (k + 1)]
        rows_hp = np.concatenate([g * HHP + jhp for g in GATE_PERM])
        whpc_cat = 0.5 * W_hh_hp[rows_hp][:, php]
        whpi_cat = W_ih_hp[rows_hp]
        js = np.arange(SS * k, SS * (k + 1))
        wsum_p = 0.5 * W_sum[js][:, php]
        woa_p = 0.5 * W_out_a[:, ja]
        wohp_p = 0.5 * W_out_hp[:, jhp]
        in_maps.append({
            "wa": _lhsT_pack(wa_cat, MA, KA).astype(bf16),
            "wsum": _lhsT_pack(wsum_p, 2, KSUM).astype(bf16),
            "whpc": _lhsT_pack(whpc_cat, MHP, KHP_C).astype(bf16),
            "whpi": _lhsT_pack(whpi_cat, MHP, KHP_I).astype(bf16),
            "woa": _lhsT_pack(woa_p, 2, 2).astype(bf16),
            "wohp": _lhsT_pack(wohp_p, 2, 4).astype(bf16),
            "baT": np.ascontiguousarray(ba_full[rows_a].reshape(MA, 128)).astype(bf16),
            "bsum2": np.ascontiguousarray((2.0 * b_sum[js]).reshape(2, 128).T),
            "bhpT": np.ascontiguousarray(bhp_full[rows_hp].reshape(MHP, 128)).astype(bf16),
            "identb": np.eye(MHP, dtype=f32).astype(bf16),
            "boa8": np.ascontiguousarray((b_out_a / NC).reshape(2, 128).T),
            "bohp8": np.ascontiguousarray((b_out_hp / NC).reshape(2, 128).T),
            "initA": initA.astype(bf16),
            "initB": initB.astype(bf16),
            "ident": np.eye(128, dtype=f32),
        })
    return in_maps


def _run(in_maps, trace=False):
    nc = _get_nc()
    return run_bass_kernel_spmd(nc, in_maps, core_ids=list(range(NC)), trace=trace)


def kernel(**inputs):
    in_maps = _prep_in_maps(**{k: np.asarray(v) for k, v in inputs.items()})
    res = _run(in_maps, trace=False)
    out = np.asarray(res.results[0]["out"], np.float32)
    return out[0][None], out[1][None]


def kernel_traced(**inputs):
    """Like kernel() but with NTFF profiling; returns ((arch, arch_hp), exec_time_ns)."""
    try:
        import ntff_hook
        ntff_hook.install()
    except Exception:
        pass
    in_maps = _prep_in_maps(**{k: np.asarray(v) for k, v in inputs.items()})
    res = _run(in_maps, trace=True)
    out = np.asarray(res.results[0]["out"], np.float32)
    return (out[0][None], out[1][None]), res.exec_time_ns


# revision 17
# speedup vs baseline: 1.5046x; 1.0044x over previous
"""Trainium2 Bass kernel for nn_ArchDecoder: two stacked LSTMs (H=2048, H=4096)
unrolled DEPTH=12 sequential steps, batch=1, tensor-parallel across 8 NeuronCores.

Schedule per step (v4):
- gather_B [c_hat|h_hat|lp_hp] launches right after the B gate math; the A-side
  (next-step arch LSTM) matmuls are fenced AFTER the launch so they execute
  during the collective's flight; gather_A [h_a_hat|lp_a] trails it.
- Scheduler-only fences (tc.no_sync_barrier) pin the phase order the Tile
  scheduler would otherwise shuffle (it has no model of collective latency).
- All sigmoids are tanh(x/2)-based so the single exp_and_others ACT table set
  serves exp+tanh+relu: zero ACT_TABLE_LOADs in the loop. States are kept 2x
  scaled with the 0.5 folded into weights host-side.
- Gate biases are folded into the PSUM accumulation via one [K=16]x identity
  matmul per LSTM, so activations read raw PSUM (no DVE bias adds).
- Weights stream over the Scalar DMA ring (payloads own the Sync ring); the
  16.8MB W_hh_hp image loads last, after everything needed early.
- Outputs accumulate in SBUF and leave via one PE-transpose + single
  contiguous DMA at the end (a per-step scatter-write stalls the DMA engines).
"""
import sys

for _p in ("/opt/trn_rl_repo", "/root/.axon_site", "/root/.axon_site/_ro/pypackages"):
    if _p not in sys.path:
        sys.path.insert(0, _p)

import numpy as np
import ml_dtypes

import concourse.bass as bass
import concourse.bacc as bacc
import concourse.mybir as mybir
import concourse.tile as tile
from concourse import bass_isa
from concourse.bass_utils import run_bass_kernel_spmd

NC = 8
V = 256
HA = 2048
HHP = 4096
DEPTH = 12
BF = mybir.dt.bfloat16
F32 = mybir.dt.float32
AF = mybir.ActivationFunctionType

SA = HA // NC          # 256 h_a positions per core
SS = HA // NC          # 256 h_sum positions per core
SHP = SA + SS          # 512 hp-state positions per core
MA = 4 * SA // 128     # 8  M-tiles for arch gates
MHP = 4 * SHP // 128   # 16 M-tiles for hp gates
KA = (V + HA) // 128   # 18 K-chunks for arch gates ([a ; h_a])
KHP_C = HHP // 128     # 32 c_hp K-chunks
KHP_I = (2 * V) // 128 # 4 inp K-chunks
KSUM = HHP // 128      # 32
CB = 10                # gather_B payload cols: [c(4) | h(4) | lpB(2)]
CHL = 6                # readback cols of the h/lp part
CA = 4                 # gather_A payload cols: [h_a(2) | lpA(2)]


def _build_nc():
    nc = bacc.Bacc(None, target_bir_lowering=False, num_devices=NC)

    wa_e = nc.declare_dram_parameter("wa", [128, MA * KA * 128], BF, isOutput=False)
    wsum_e = nc.declare_dram_parameter("wsum", [128, 2 * KSUM * 128], BF, isOutput=False)
    whpc_e = nc.declare_dram_parameter("whpc", [128, MHP * KHP_C * 128], BF, isOutput=False)
    whpi_e = nc.declare_dram_parameter("whpi", [128, MHP * KHP_I * 128], BF, isOutput=False)
    woa_e = nc.declare_dram_parameter("woa", [128, 2 * 2 * 128], BF, isOutput=False)
    wohp_e = nc.declare_dram_parameter("wohp", [128, 2 * 4 * 128], BF, isOutput=False)
    baT_e = nc.declare_dram_parameter("baT", [MA, 128], BF, isOutput=False)
    bsum2_e = nc.declare_dram_parameter("bsum2", [128, 2], F32, isOutput=False)
    bhpT_e = nc.declare_dram_parameter("bhpT", [MHP, 128], BF, isOutput=False)
    identb_e = nc.declare_dram_parameter("identb", [MHP, MHP], BF, isOutput=False)
    boa8_e = nc.declare_dram_parameter("boa8", [128, 2], F32, isOutput=False)
    bohp8_e = nc.declare_dram_parameter("bohp8", [128, 2], F32, isOutput=False)
    initA_e = nc.declare_dram_parameter("initA", [128, NC * CA], BF, isOutput=False)
    initB_e = nc.declare_dram_parameter("initB", [128, NC * CHL], BF, isOutput=False)
    ident_e = nc.declare_dram_parameter("ident", [128, 128], F32, isOutput=False)
    out_e = nc.declare_dram_parameter("out", [2, DEPTH, V], F32, isOutput=True)

    with tile.TileContext(nc, num_cores=NC) as tc:
        with (
            tc.tile_pool(name="wpool", bufs=1) as wpool,
            tc.tile_pool(name="cpool", bufs=1) as cpool,
            tc.tile_pool(name="spool", bufs=3) as spool,
            tc.tile_pool(name="xpool", bufs=3) as xpool,
            tc.tile_pool(name="psA", bufs=2, space="PSUM") as psA,
            tc.tile_pool(name="psHP", bufs=1, space="PSUM") as psHP,
            tc.tile_pool(name="psM", bufs=2, space="PSUM") as psM,
            tc.tile_pool(name="dram", bufs=2, space="DRAM") as dram,
        ):
            # --- weight/const tiles; DMA emission order == load order ---
            initA0 = xpool.tile([128, NC * CA], BF, tag="allA", bufs=3)
            initB0 = xpool.tile([128, NC * CHL], BF, tag="allBhl", bufs=3)
            baT = cpool.tile([MA, 128], BF, tag="baT")
            bhpT = cpool.tile([MHP, 128], BF, tag="bhpT")
            identb = cpool.tile([MHP, MHP], BF, tag="identb")
            wa = wpool.tile([128, MA * KA * 128], BF, tag="wa")
            bsum2 = cpool.tile([128, 2], F32, tag="bsum2")
            wsum = wpool.tile([128, 2 * KSUM * 128], BF, tag="wsum")
            whpi = wpool.tile([128, MHP * KHP_I * 128], BF, tag="whpi")
            woa = wpool.tile([128, 2 * 2 * 128], BF, tag="woa")
            wohp = wpool.tile([128, 2 * 4 * 128], BF, tag="wohp")
            boa8 = cpool.tile([128, 2], F32, tag="boa8")
            bohp8 = cpool.tile([128, 2], F32, tag="bohp8")
            whpc = wpool.tile([128, MHP * KHP_C * 128], BF, tag="whpc")
            ident = cpool.tile([128, 128], F32, tag="ident")
            # weights go through the Scalar engine's DMA ring so the per-step
            # payload/readback DMAs (on the Sync ring) aren't queued behind 26MB
            nc.scalar.dma_start(initA0[:], initA_e[:])
            nc.scalar.dma_start(initB0[:], initB_e[:])
            nc.scalar.dma_start(baT[:], baT_e[:])
            nc.scalar.dma_start(bhpT[:], bhpT_e[:])
            nc.scalar.dma_start(identb[:], identb_e[:])
            nc.scalar.dma_start(wa[:], wa_e[:])
            nc.scalar.dma_start(bsum2[:], bsum2_e[:])
            nc.scalar.dma_start(wsum[:], wsum_e[:])
            nc.scalar.dma_start(whpi[:], whpi_e[:])
            nc.scalar.dma_start(woa[:], woa_e[:])
            nc.scalar.dma_start(wohp[:], wohp_e[:])
            nc.scalar.dma_start(boa8[:], boa8_e[:])
            nc.scalar.dma_start(bohp8[:], bohp8_e[:])
            nc.scalar.dma_start(ident[:], ident_e[:])
            nc.scalar.dma_start(whpc[:], whpc_e[:])

            c2_a = cpool.tile([128, 2], F32, tag="c2_a")   # 2*c_a state
            nc.vector.memset(c2_a[:], 0.0)
            # outputs accumulate on-chip; [0:24]=arch, [24:48]=arch_hp
            outAB = cpool.tile([128, 4 * DEPTH], F32, tag="outAB")
            outA = outAB[:, 0:2 * DEPTH]
            outHP = outAB[:, 2 * DEPTH:4 * DEPTH]

            a_bf0 = xpool.tile([128, 2], BF, tag="a_bf")
            ahp_bf0 = xpool.tile([128, 2], BF, tag="ahp_bf")
            nc.vector.memset(a_bf0[:], 1.0 / V)
            nc.vector.memset(ahp_bf0[:], 1.0 / V)

            # column helpers: allB_c[:, kc] is c_hat chunk kc directly;
            # h/lp live in allB_hl with rank stride CHL
            hcol = lambda kc: (kc // 4) * CHL + (kc % 4)
            acol = lambda j: (j // 2) * CA + (j % 2)

            def A_step(a_bf, allA):
                """arch LSTM step: bias matmul + gate matvecs + gate math + payA."""
                ga_ps = psA.tile([128, MA], F32, tag="ga_ps")
                nc.tensor.matmul(ga_ps[:, 0:MA], baT[:], identb[0:MA, 0:MA],
                                 start=True, stop=False)
                for m in range(MA):
                    for kc in list(range(2, KA)) + [0, 1]:
                        rhs = a_bf[:, kc:kc + 1] if kc < 2 else allA[:, acol(kc - 2):acol(kc - 2) + 1]
                        nc.tensor.matmul(
                            ga_ps[:, m:m + 1],
                            wa[:, (m * KA + kc) * 128:(m * KA + kc + 1) * 128],
                            rhs, start=False, stop=(kc == 1),
                        )
                acts = spool.tile([128, MA], F32, tag="acts_a")
                # gate layout [i(0:2) f(2:4) o(4:6) g(6:8)]
                nc.scalar.activation(acts[:, 0:6], ga_ps[:, 0:6], AF.Tanh, scale=0.5)
                nc.scalar.activation(acts[:, 6:8], ga_ps[:, 6:8], AF.Tanh)
                m1 = spool.tile([128, 2], F32, tag="am1")
                s1 = spool.tile([128, 2], F32, tag="as1")
                nc.vector.tensor_mul(m1[:], acts[:, 0:2], acts[:, 6:8])
                nc.vector.tensor_add(s1[:], acts[:, 6:8], m1[:])
                m2 = spool.tile([128, 2], F32, tag="am2")
                s2 = spool.tile([128, 2], F32, tag="as2")
                nc.vector.tensor_mul(m2[:], acts[:, 2:4], c2_a[:])
                nc.vector.tensor_add(s2[:], c2_a[:], m2[:])
                s2h = spool.tile([128, 2], F32, tag="as2h")
                nc.vector.tensor_scalar_mul(s2h[:], s2[:], 0.5)
                nc.vector.tensor_add(c2_a[:], s1[:], s2h[:])
                tc_a = spool.tile([128, 2], F32, tag="tc_a")
                nc.scalar.activation(tc_a[:], c2_a[:], AF.Tanh, scale=0.5)
                ph2 = spool.tile([128, 4], F32, tag="ph2")
                m3 = spool.tile([128, 2], F32, tag="am3")
                nc.vector.tensor_mul(m3[:], acts[:, 4:6], tc_a[:])
                nc.vector.tensor_add(ph2[:, 0:2], tc_a[:], m3[:])
                payA = spool.tile([128, CA], BF, tag="payA")
                nc.vector.tensor_copy(payA[:, 0:2], ph2[:, 0:2])
                la_ps = psM.tile([128, 2], F32, tag="psM")
                for m in range(2):
                    for kc in range(2):
                        nc.tensor.matmul(
                            la_ps[:, m:m + 1],
                            woa[:, (m * 2 + kc) * 128:(m * 2 + kc + 1) * 128],
                            payA[:, kc:kc + 1], start=(kc == 0), stop=(kc == 1),
                        )
                nc.vector.tensor_add(payA[:, 2:4], la_ps[:], boa8[:])
                return ph2, payA

            def gather(pay, cols, tagc, tagg):
                cc = dram.tile([128, cols], BF, tag=tagc)
                g = dram.tile([NC, 128, cols], BF, tag=tagg)
                nc.sync.dma_start(cc[:], pay[:])
                nc.gpsimd.collective_compute(
                    "AllGather", mybir.AluOpType.bypass,
                    replica_groups=[list(range(NC))],
                    ins=[cc.opt()], outs=[g.opt()],
                )
                return g

            def softmax(allT, lo, t, outT, tag):
                """reduce rank partials -> exp -> normalize; returns prob bf16."""
                red = spool.tile([128, 2], F32, tag=f"red{tag}")
                nc.vector.tensor_reduce(
                    red[:],
                    allT[:].rearrange("p (r c) -> p c r", r=NC)[:, lo:lo + 2, :],
                    mybir.AxisListType.X, mybir.AluOpType.add,
                )
                ex = spool.tile([128, 2], F32, tag=f"ex{tag}")
                sfree = spool.tile([128, 1], F32, tag=f"sf{tag}")
                nc.scalar.activation(ex[:], red[:], AF.Exp, accum_out=sfree[:])
                spart = spool.tile([128, 1], F32, tag=f"sp{tag}")
                nc.gpsimd.partition_all_reduce(spart[:], sfree[:], 128, bass_isa.ReduceOp.add)
                zinv = spool.tile([128, 1], F32, tag=f"zi{tag}")
                nc.vector.reciprocal(zinv[:], spart[:])
                nc.vector.tensor_scalar_mul(outT[:, 2 * t:2 * t + 2], ex[:], zinv[:, 0:1])
                prob = xpool.tile([128, 2], BF, tag=f"{tag}_bf")
                nc.vector.tensor_copy(prob[:], outT[:, 2 * t:2 * t + 2])
                return prob

            # ---------------- preamble: A-step 0 + its gather ----------------
            ph2_cur, payA0 = A_step(a_bf0, initA0)
            gA = gather(payA0, CA, "ccA", "gA")
            allA_cur = xpool.tile([128, NC * CA], BF, tag="allA")
            nc.sync.dma_start(allA_cur[:], gA[:].rearrange("r p c -> p r c"))

            allB_c = None
            allB_hl = initB0
            ahp_bf = ahp_bf0

            for t in range(DEPTH):
                # softmaxes first; fenced ahead of the B-side DVE work
                if t > 0:
                    ahp_bf = softmax(allB_hl, 4, t - 1, outHP, "ahp")
                a_bf = softmax(allA_cur, 2, t, outA, "a")
                tc.no_sync_barrier()

                # --- B-side MMs: one PSUM accumulation [bias | W_hh@c | W_ih@inp]
                ghp_ps = psHP.tile([128, MHP], F32, tag="ghp_ps")
                nc.tensor.matmul(ghp_ps[:, 0:MHP], bhpT[:], identb[:],
                                 start=True, stop=False)
                if t > 0:
                    for m in range(MHP):
                        for kc in range(KHP_C):
                            nc.tensor.matmul(
                                ghp_ps[:, m:m + 1],
                                whpc[:, (m * KHP_C + kc) * 128:(m * KHP_C + kc + 1) * 128],
                                allB_c[:, kc:kc + 1],
                                start=False, stop=False,
                            )
                for m in range(MHP):
                    for j in range(KHP_I):
                        rhs = a_bf[:, j:j + 1] if j < 2 else ahp_bf[:, j - 2:j - 1]
                        nc.tensor.matmul(
                            ghp_ps[:, m:m + 1],
                            whpi[:, (m * KHP_I + j) * 128:(m * KHP_I + j + 1) * 128],
                            rhs, start=False, stop=(j == KHP_I - 1),
                        )
                hs_ps = psM.tile([128, 2], F32, tag="psM")
                for m in range(2):
                    for kc in range(KSUM):
                        nc.tensor.matmul(
                            hs_ps[:, m:m + 1],
                            wsum[:, (m * KSUM + kc) * 128:(m * KSUM + kc + 1) * 128],
                            allB_hl[:, hcol(kc):hcol(kc) + 1],
                            start=(kc == 0), stop=(kc == KSUM - 1),
                        )
                # h_sum_hat = 2*relu(W_sum@h + b_sum) = relu(2*psum + 2*b_sum)
                nc.scalar.activation(ph2_cur[:, 2:3], hs_ps[:, 0:1], AF.Relu,
                                     bias=bsum2[:, 0:1], scale=2.0)
                nc.scalar.activation(ph2_cur[:, 3:4], hs_ps[:, 1:2], AF.Relu,
                                     bias=bsum2[:, 1:2], scale=2.0)

                # --- B-side gate math (DVE/ACT); gates read PSUM directly ---
                acts_h = spool.tile([128, MHP], F32, tag="acts_h")
                # gate layout [i(0:4) f(4:8) o(8:12) g(12:16)]
                nc.scalar.activation(acts_h[:, 0:12], ghp_ps[:, 0:12], AF.Tanh, scale=0.5)
                nc.scalar.activation(acts_h[:, 12:16], ghp_ps[:, 12:16], AF.Tanh)
                hm1 = spool.tile([128, 4], F32, tag="hm1")
                hs1 = spool.tile([128, 4], F32, tag="hs1")
                nc.vector.tensor_mul(hm1[:], acts_h[:, 0:4], acts_h[:, 12:16])
                nc.vector.tensor_add(hs1[:], acts_h[:, 12:16], hm1[:])
                hm2 = spool.tile([128, 4], F32, tag="hm2")
                hs2 = spool.tile([128, 4], F32, tag="hs2")
                nc.vector.tensor_mul(hm2[:], acts_h[:, 4:8], ph2_cur[:])
                nc.vector.tensor_add(hs2[:], ph2_cur[:], hm2[:])
                hs2h = spool.tile([128, 4], F32, tag="hs2h")
                nc.vector.tensor_scalar_mul(hs2h[:], hs2[:], 0.5)
                payB = spool.tile([128, CB], BF, tag="payB")
                nc.vector.tensor_add(payB[:, 0:4], hs1[:], hs2h[:])   # c_hat (bf16)
                tch = spool.tile([128, 4], F32, tag="tch")
                nc.scalar.activation(tch[:], payB[:, 0:4], AF.Tanh, scale=0.5)
                hm3 = spool.tile([128, 4], F32, tag="hm3")
                nc.vector.tensor_mul(hm3[:], acts_h[:, 8:12], tch[:])
                nc.vector.tensor_add(payB[:, 4:8], tch[:], hm3[:])    # h_hat (bf16)

                lhp_ps = psM.tile([128, 2], F32, tag="psM")
                for m in range(2):
                    for kc in range(4):
                        nc.tensor.matmul(
                            lhp_ps[:, m:m + 1],
                            wohp[:, (m * 4 + kc) * 128:(m * 4 + kc + 1) * 128],
                            payB[:, 4 + kc:5 + kc], start=(kc == 0), stop=(kc == 3),
                        )
                nc.vector.tensor_add(payB[:, 8:10], lhp_ps[:], bohp8[:])

                gB = gather(payB, CB, "ccB", "gB")
                tc.no_sync_barrier()

                if t + 1 < DEPTH:
                    ph2_next, payA = A_step(a_bf, allA_cur)
                    gA = gather(payA, CA, "ccA", "gA")

                # c-part readback split by rank halves: the W_hh@c chunks for
                # ranks 0-3 only wait on the first half
                allB_c = xpool.tile([128, NC * 4], BF, tag="allBc")
                nc.sync.dma_start(allB_c[:, 0:16], gB[0:4, :, 0:4].rearrange("r p c -> p r c"))
                nc.sync.dma_start(allB_c[:, 16:32], gB[4:NC, :, 0:4].rearrange("r p c -> p r c"))
                allB_hl = xpool.tile([128, NC * CHL], BF, tag="allBhl")
                nc.sync.dma_start(allB_hl[:], gB[:, :, 4:CB].rearrange("r p c -> p r c"))
                if t + 1 < DEPTH:
                    allA_cur = xpool.tile([128, NC * CA], BF, tag="allA")
                    nc.sync.dma_start(allA_cur[:], gA[:].rearrange("r p c -> p r c"))
                    ph2_cur = ph2_next

            # tail: last hp softmax
            softmax(allB_hl, 4, DEPTH - 1, outHP, "ahp")

            # transpose [128, 48] -> [48, 128] on PE so the output DMA writes
            # contiguous 512B rows instead of a 4-byte-packet scatter
            tr_ps = psM.tile([4 * DEPTH, 128], F32, tag="tr_ps", bufs=1)
            nc.tensor.transpose(tr_ps[:], outAB[:], ident[:])
            trf = spool.tile([4 * DEPTH, 128], F32, tag="trf", bufs=1)
            nc.vector.tensor_copy(trf[:], tr_ps[:])
            nc.sync.dma_start(
                out_e[:].rearrange("s t (m p) -> (s t m) p", p=128),
                trf[:],
            )
    nc.finalize()
    return nc


_NC_CACHE = None


def _get_nc():
    global _NC_CACHE
    if _NC_CACHE is None:
        _NC_CACHE = _build_nc()
    return _NC_CACHE


def _lhsT_pack(w_cat, n_m, n_k):
    """w_cat [n_m*128 rows, n_k*128 cols] -> SBUF image [128, n_m*n_k*128] where
    cols [(m*n_k+kc)*128 + j] on partition p = w_cat[m*128 + j, kc*128 + p]."""
    a = w_cat.reshape(n_m, 128, n_k, 128)           # [m, j, kc, p]
    return np.ascontiguousarray(a.transpose(3, 0, 2, 1).reshape(128, n_m * n_k * 128))


GATE_PERM = (0, 1, 3, 2)  # pytorch [i,f,g,o] -> kernel [i,f,o,g]


def _prep_in_maps(x_thought_vec_arch, x_thought_vec_arch_hp,
                  W_ih_a, W_hh_a, b_ih_a, b_hh_a, W_out_a, b_out_a,
                  W_sum, b_sum, W_ih_hp, W_hh_hp, b_ih_hp, b_hh_hp,
                  W_out_hp, b_out_hp):
    f32 = np.float32
    bf16 = ml_dtypes.bfloat16
    php = np.concatenate([
        np.concatenate([np.arange(SA * k, SA * (k + 1)),
                        HA + np.arange(SS * k, SS * (k + 1))])
        for k in range(NC)
    ])
    ba_full = (np.asarray(b_ih_a) + np.asarray(b_hh_a)).astype(f32)
    bhp_full = (np.asarray(b_ih_hp) + np.asarray(b_hh_hp)).astype(f32)
    ha0 = np.asarray(x_thought_vec_arch, f32).reshape(HA)
    hhp0 = np.asarray(x_thought_vec_arch_hp, f32).reshape(HHP)
    W_ih_a = np.asarray(W_ih_a, f32); W_hh_a = np.asarray(W_hh_a, f32)
    W_out_a = np.asarray(W_out_a, f32); W_sum = np.asarray(W_sum, f32)
    W_ih_hp = np.asarray(W_ih_hp, f32); W_hh_hp = np.asarray(W_hh_hp, f32)
    W_out_hp = np.asarray(W_out_hp, f32)
    b_out_a = np.asarray(b_out_a, f32); b_out_hp = np.asarray(b_out_hp, f32)
    b_sum = np.asarray(b_sum, f32)

    # init images in "2x" space, laid out like the gathered tiles
    initA = np.zeros((128, NC * CA), f32)
    initB = np.zeros((128, NC * CHL), f32)
    ha0_2 = 2.0 * ha0
    hhp0_2 = (2.0 * hhp0)[php]
    for r in range(NC):
        for j in range(2):
            initA[:, r * CA + j] = ha0_2[r * SA + j * 128: r * SA + (j + 1) * 128]
        for q in range(4):
            initB[:, r * CHL + q] = hhp0_2[r * SHP + q * 128: r * SHP + (q + 1) * 128]

    in_maps = []
    for k in range(NC):
        ja = np.arange(SA * k, SA * (k + 1))
        rows_a = np.concatenate([g * HA + ja for g in GATE_PERM])
        wa_cat = np.concatenate([W_ih_a[rows_a], 0.5 * W_hh_a[rows_a]], axis=1)
        jhp = php[SHP * k: SHP * (k + 1)]
        rows_hp = np.concatenate([g * HHP + jhp for g in GATE_PERM])
        whpc_cat = 0.5 * W_hh_hp[rows_hp][:, php]
        whpi_cat = W_ih_hp[rows_hp]
        js = np.arange(SS * k, SS * (k + 1))
        wsum_p = 0.5 * W_sum[js][:, php]
        woa_p = 0.5 * W_out_a[:, ja]
        wohp_p = 0.5 * W_out_hp[:, jhp]
        in_maps.append({
            "wa": _lhsT_pack(wa_cat, MA, KA).astype(bf16),
            "wsum": _lhsT_pack(wsum_p, 2, KSUM).astype(bf16),
            "whpc": _lhsT_pack(whpc_cat, MHP, KHP_C).astype(bf16),
            "whpi": _lhsT_pack(whpi_cat, MHP, KHP_I).astype(bf16),
            "woa": _lhsT_pack(woa_p, 2, 2).astype(bf16),
            "wohp": _lhsT_pack(wohp_p, 2, 4).astype(bf16),
            "baT": np.ascontiguousarray(ba_full[rows_a].reshape(MA, 128)).astype(bf16),
            "bsum2": np.ascontiguousarray((2.0 * b_sum[js]).reshape(2, 128).T),
            "bhpT": np.ascontiguousarray(bhp_full[rows_hp].reshape(MHP, 128)).astype(bf16),
            "identb": np.eye(MHP, dtype=f32).astype(bf16),
            "boa8": np.ascontiguousarray((b_out_a / NC).reshape(2, 128).T),
            "bohp8": np.ascontiguousarray((b_out_hp / NC).reshape(2, 128).T),
            "initA": initA.astype(bf16),
            "initB": initB.astype(bf16),
            "ident": np.eye(128, dtype=f32),
        })
    return in_maps


def _run(in_maps, trace=False):
    nc = _get_nc()
    return run_bass_kernel_spmd(nc, in_maps, core_ids=list(range(NC)), trace=trace)


def kernel(**inputs):
    in_maps = _prep_in_maps(**{k: np.asarray(v) for k, v in inputs.items()})
    res = _run(in_maps, trace=False)
    out = np.asarray(res.results[0]["out"], np.float32)
    return out[0][None], out[1][None]


def kernel_traced(**inputs):
    """Like kernel() but with NTFF profiling; returns ((arch, arch_hp), exec_time_ns)."""
    try:
        import ntff_hook
        ntff_hook.install()
    except Exception:
        pass
    in_maps = _prep_in_maps(**{k: np.asarray(v) for k, v in inputs.items()})
    res = _run(in_maps, trace=True)
    out = np.asarray(res.results[0]["out"], np.float32)
    return (out[0][None], out[1][None]), res.exec_time_ns


# revision 25
# speedup vs baseline: 1.5786x; 1.0492x over previous
"""Trainium2 Bass kernel for nn_ArchDecoder: two stacked LSTMs (H=2048, H=4096)
unrolled DEPTH=12 sequential steps, batch=1, tensor-parallel across 8 NeuronCores.

Schedule per step (v4):
- gather_B [c_hat|h_hat|lp_hp] launches right after the B gate math; the A-side
  (next-step arch LSTM) matmuls are fenced AFTER the launch so they execute
  during the collective's flight; gather_A [h_a_hat|lp_a] trails it.
- Scheduler-only fences (tc.no_sync_barrier) pin the phase order the Tile
  scheduler would otherwise shuffle (it has no model of collective latency).
- All sigmoids are tanh(x/2)-based so the single exp_and_others ACT table set
  serves exp+tanh+relu: zero ACT_TABLE_LOADs in the loop. States are kept 2x
  scaled with the 0.5 folded into weights host-side.
- Gate biases are folded into the PSUM accumulation via one [K=16]x identity
  matmul per LSTM, so activations read raw PSUM (no DVE bias adds).
- Weights stream over the Scalar DMA ring (payloads own the Sync ring); the
  16.8MB W_hh_hp image loads last, after everything needed early.
- Outputs accumulate in SBUF and leave via one PE-transpose + single
  contiguous DMA at the end (a per-step scatter-write stalls the DMA engines).
"""
import sys

for _p in ("/opt/trn_rl_repo", "/root/.axon_site", "/root/.axon_site/_ro/pypackages"):
    if _p not in sys.path:
        sys.path.insert(0, _p)

import numpy as np
import ml_dtypes

import concourse.bass as bass
import concourse.bacc as bacc
import concourse.mybir as mybir
import concourse.tile as tile
from concourse import bass_isa
from concourse.bass_utils import run_bass_kernel_spmd

NC = 8
V = 256
HA = 2048
HHP = 4096
DEPTH = 12
BF = mybir.dt.bfloat16
F32 = mybir.dt.float32
FP8 = mybir.dt.float8e4
AF = mybir.ActivationFunctionType
WSC = 256.0   # fp8 scale for W_hh_hp; compensated by c_hat/WSC in the payload

SA = HA // NC          # 256 h_a positions per core
SS = HA // NC          # 256 h_sum positions per core
SHP = SA + SS          # 512 hp-state positions per core
MA = 4 * SA // 128     # 8  M-tiles for arch gates
MHP = 4 * SHP // 128   # 16 M-tiles for hp gates
KA = (V + HA) // 128   # 18 K-chunks for arch gates ([a ; h_a])
KHP_C = HHP // 128     # 32 c_hp K-chunks
KHP_I = (2 * V) // 128 # 4 inp K-chunks
KSUM = HHP // 128      # 32
CB = 10                # gather_B payload cols: [c(4) | h(4) | lpB(2)]
CHL = 6                # readback cols of the h/lp part
CA = 4                 # gather_A payload cols: [h_a(2) | lpA(2)]


def _build_nc():
    nc = bacc.Bacc(None, target_bir_lowering=False, num_devices=NC)

    wa_e = nc.declare_dram_parameter("wa", [128, MA * KA * 128], BF, isOutput=False)
    wsum_e = nc.declare_dram_parameter("wsum", [128, 2 * KSUM * 128], BF, isOutput=False)
    whpc_e = nc.declare_dram_parameter("whpc", [128, MHP * KHP_C * 128], FP8, isOutput=False)
    whpi_e = nc.declare_dram_parameter("whpi", [128, MHP * KHP_I * 128], BF, isOutput=False)
    woa_e = nc.declare_dram_parameter("woa", [128, 2 * 2 * 128], BF, isOutput=False)
    wohp_e = nc.declare_dram_parameter("wohp", [128, 2 * 4 * 128], BF, isOutput=False)
    baT_e = nc.declare_dram_parameter("baT", [MA, 128], BF, isOutput=False)
    bsum2_e = nc.declare_dram_parameter("bsum2", [128, 2], F32, isOutput=False)
    bhpT_e = nc.declare_dram_parameter("bhpT", [MHP, 128], BF, isOutput=False)
    identb_e = nc.declare_dram_parameter("identb", [MHP, MHP], BF, isOutput=False)
    boa8_e = nc.declare_dram_parameter("boa8", [128, 2], F32, isOutput=False)
    bohp8_e = nc.declare_dram_parameter("bohp8", [128, 2], F32, isOutput=False)
    initA_e = nc.declare_dram_parameter("initA", [128, NC * CA], BF, isOutput=False)
    initB_e = nc.declare_dram_parameter("initB", [128, NC * CHL], BF, isOutput=False)
    ident_e = nc.declare_dram_parameter("ident", [128, 128], F32, isOutput=False)
    out_e = nc.declare_dram_parameter("out", [2, DEPTH, V], F32, isOutput=True)

    with tile.TileContext(nc, num_cores=NC) as tc:
        with (
            tc.tile_pool(name="wpool", bufs=1) as wpool,
            tc.tile_pool(name="cpool", bufs=1) as cpool,
            tc.tile_pool(name="spool", bufs=3) as spool,
            tc.tile_pool(name="xpool", bufs=3) as xpool,
            tc.tile_pool(name="psA", bufs=2, space="PSUM") as psA,
            tc.tile_pool(name="psHP", bufs=1, space="PSUM") as psHP,
            tc.tile_pool(name="psM", bufs=2, space="PSUM") as psM,
            tc.tile_pool(name="dram", bufs=2, space="DRAM") as dram,
        ):
            # --- weight/const tiles; DMA emission order == load order ---
            initA0 = xpool.tile([128, NC * CA], BF, tag="allA", bufs=3)
            initB0 = xpool.tile([128, NC * CHL], BF, tag="allBhl", bufs=3)
            baT = cpool.tile([MA, 128], BF, tag="baT")
            bhpT = cpool.tile([MHP, 128], BF, tag="bhpT")
            identb = cpool.tile([MHP, MHP], BF, tag="identb")
            wa = wpool.tile([128, MA * KA * 128], BF, tag="wa")
            bsum2 = cpool.tile([128, 2], F32, tag="bsum2")
            wsum = wpool.tile([128, 2 * KSUM * 128], BF, tag="wsum")
            whpi = wpool.tile([128, MHP * KHP_I * 128], BF, tag="whpi")
            woa = wpool.tile([128, 2 * 2 * 128], BF, tag="woa")
            wohp = wpool.tile([128, 2 * 4 * 128], BF, tag="wohp")
            boa8 = cpool.tile([128, 2], F32, tag="boa8")
            bohp8 = cpool.tile([128, 2], F32, tag="bohp8")
            whpc = wpool.tile([128, MHP * KHP_C * 128], FP8, tag="whpc")
            ident = cpool.tile([128, 128], F32, tag="ident")
            # weights go through the Scalar engine's DMA ring so the per-step
            # payload/readback DMAs (on the Sync ring) aren't queued behind 26MB
            nc.scalar.dma_start(initA0[:], initA_e[:])
            nc.scalar.dma_start(initB0[:], initB_e[:])
            nc.scalar.dma_start(baT[:], baT_e[:])
            nc.scalar.dma_start(bhpT[:], bhpT_e[:])
            nc.scalar.dma_start(identb[:], identb_e[:])
            # wa in m-tile chunks so A-step 0 starts as soon as tile 0 lands
            WCH = KA * 128
            for m in range(MA):
                nc.scalar.dma_start(wa[:, m * WCH:(m + 1) * WCH],
                                    wa_e[:, m * WCH:(m + 1) * WCH])
            nc.scalar.dma_start(bsum2[:], bsum2_e[:])
            nc.scalar.dma_start(wsum[:], wsum_e[:])
            nc.scalar.dma_start(whpi[:], whpi_e[:])
            nc.scalar.dma_start(woa[:], woa_e[:])
            nc.scalar.dma_start(wohp[:], wohp_e[:])
            nc.scalar.dma_start(boa8[:], boa8_e[:])
            nc.scalar.dma_start(bohp8[:], bohp8_e[:])
            nc.scalar.dma_start(ident[:], ident_e[:])
            nc.scalar.dma_start(whpc[:], whpc_e[:])

            c2_a = cpool.tile([128, 2], F32, tag="c2_a")   # 2*c_a state
            nc.vector.memset(c2_a[:], 0.0)
            # outputs accumulate on-chip; [0:24]=arch, [24:48]=arch_hp
            outAB = cpool.tile([128, 4 * DEPTH], F32, tag="outAB")
            outA = outAB[:, 0:2 * DEPTH]
            outHP = outAB[:, 2 * DEPTH:4 * DEPTH]

            a_bf0 = xpool.tile([128, 2], BF, tag="a_bf")
            ahp_bf0 = xpool.tile([128, 2], BF, tag="ahp_bf")
            nc.vector.memset(a_bf0[:], 1.0 / V)
            nc.vector.memset(ahp_bf0[:], 1.0 / V)

            # warm the PE (HAM un-throttles after ~3.4us of activity) while
            # the weight DMAs stream
            wu_ps = psM.tile([32, 1], F32, tag="psM")
            for _ in range(40):
                nc.tensor.matmul(wu_ps[:], initA0[:, 0:32], a_bf0[:, 0:1],
                                 start=True, stop=True)

            # column helpers: allB_c[:, kc] is c_hat chunk kc directly;
            # h/lp live in allB_hl with rank stride CHL
            hcol = lambda kc: (kc // 4) * CHL + (kc % 4)
            acol = lambda j: (j // 2) * CA + (j % 2)

            def A_step(a_bf, allA):
                """arch LSTM step: bias matmul + gate matvecs + gate math + payA."""
                ga_ps = psA.tile([128, MA], F32, tag="ga_ps")
                nc.tensor.matmul(ga_ps[:, 0:MA], baT[:], identb[0:MA, 0:MA],
                                 start=True, stop=False)
                for m in range(MA):
                    for kc in list(range(2, KA)) + [0, 1]:
                        rhs = a_bf[:, kc:kc + 1] if kc < 2 else allA[:, acol(kc - 2):acol(kc - 2) + 1]
                        nc.tensor.matmul(
                            ga_ps[:, m:m + 1],
                            wa[:, (m * KA + kc) * 128:(m * KA + kc + 1) * 128],
                            rhs, start=False, stop=(kc == 1),
                        )
                acts = spool.tile([128, MA], F32, tag="acts_a")
                # gate layout [i(0:2) f(2:4) o(4:6) g(6:8)]
                nc.scalar.activation(acts[:, 0:6], ga_ps[:, 0:6], AF.Tanh, scale=0.5)
                nc.scalar.activation(acts[:, 6:8], ga_ps[:, 6:8], AF.Tanh)
                m1 = spool.tile([128, 2], F32, tag="am1")
                s1 = spool.tile([128, 2], F32, tag="as1")
                nc.vector.tensor_mul(m1[:], acts[:, 0:2], acts[:, 6:8])
                nc.vector.tensor_add(s1[:], acts[:, 6:8], m1[:])
                m2 = spool.tile([128, 2], F32, tag="am2")
                s2 = spool.tile([128, 2], F32, tag="as2")
                nc.vector.tensor_mul(m2[:], acts[:, 2:4], c2_a[:])
                nc.vector.tensor_add(s2[:], c2_a[:], m2[:])
                s2h = spool.tile([128, 2], F32, tag="as2h")
                nc.vector.tensor_scalar_mul(s2h[:], s2[:], 0.5)
                nc.vector.tensor_add(c2_a[:], s1[:], s2h[:])
                tc_a = spool.tile([128, 2], F32, tag="tc_a")
                nc.scalar.activation(tc_a[:], c2_a[:], AF.Tanh, scale=0.5)
                ph2 = spool.tile([128, 4], F32, tag="ph2")
                m3 = spool.tile([128, 2], F32, tag="am3")
                nc.vector.tensor_mul(m3[:], acts[:, 4:6], tc_a[:])
                nc.vector.tensor_add(ph2[:, 0:2], tc_a[:], m3[:])
                payA = spool.tile([128, CA], BF, tag="payA")
                nc.vector.tensor_copy(payA[:, 0:2], ph2[:, 0:2])
                la_ps = psM.tile([128, 2], F32, tag="psM")
                for m in range(2):
                    for kc in range(2):
                        nc.tensor.matmul(
                            la_ps[:, m:m + 1],
                            woa[:, (m * 2 + kc) * 128:(m * 2 + kc + 1) * 128],
                            payA[:, kc:kc + 1], start=(kc == 0), stop=(kc == 1),
                        )
                nc.vector.tensor_add(payA[:, 2:4], la_ps[:], boa8[:])
                return ph2, payA

            def gather(pay, cols, tagc, tagg):
                cc = dram.tile([128, cols], BF, tag=tagc)
                g = dram.tile([NC, 128, cols], BF, tag=tagg)
                nc.sync.dma_start(cc[:], pay[:])
                nc.gpsimd.collective_compute(
                    "AllGather", mybir.AluOpType.bypass,
                    replica_groups=[list(range(NC))],
                    ins=[cc.opt()], outs=[g.opt()],
                )
                return g

            def softmax(allT, lo, t, outT, tag):
                """reduce rank partials -> exp -> normalize; returns prob bf16."""
                red = spool.tile([128, 2], F32, tag=f"red{tag}")
                nc.vector.tensor_reduce(
                    red[:],
                    allT[:].rearrange("p (r c) -> p c r", r=NC)[:, lo:lo + 2, :],
                    mybir.AxisListType.X, mybir.AluOpType.add,
                )
                ex = spool.tile([128, 2], F32, tag=f"ex{tag}")
                sfree = spool.tile([128, 1], F32, tag=f"sf{tag}")
                nc.scalar.activation(ex[:], red[:], AF.Exp, accum_out=sfree[:])
                spart = spool.tile([128, 1], F32, tag=f"sp{tag}")
                nc.gpsimd.partition_all_reduce(spart[:], sfree[:], 128, bass_isa.ReduceOp.add)
                zinv = spool.tile([128, 1], F32, tag=f"zi{tag}")
                nc.vector.reciprocal(zinv[:], spart[:])
                nc.vector.tensor_scalar_mul(outT[:, 2 * t:2 * t + 2], ex[:], zinv[:, 0:1])
                prob = xpool.tile([128, 2], BF, tag=f"{tag}_bf")
                nc.vector.tensor_copy(prob[:], outT[:, 2 * t:2 * t + 2])
                return prob

            # ---------------- preamble: A-step 0 + its gather ----------------
            ph2_cur, payA0 = A_step(a_bf0, initA0)
            gA = gather(payA0, CA, "ccA", "gA")
            allA_cur = xpool.tile([128, NC * CA], BF, tag="allA")
            nc.sync.dma_start(allA_cur[:], gA[:].rearrange("r p c -> p r c"))

            allB_c = None
            allB_hl = initB0
            ahp_bf = ahp_bf0

            for t in range(DEPTH):
                # softmaxes first; fenced ahead of the B-side DVE work
                if t > 0:
                    ahp_bf = softmax(allB_hl, 4, t - 1, outHP, "ahp")
                a_bf = softmax(allA_cur, 2, t, outA, "a")
                tc.no_sync_barrier()

                # --- B-side MMs: one PSUM accumulation [bias | W_hh@c | W_ih@inp]
                ghp_ps = psHP.tile([128, MHP], F32, tag="ghp_ps")
                nc.tensor.matmul(ghp_ps[:, 0:MHP], bhpT[:], identb[:],
                                 start=True, stop=False)
                if t > 0:
                    for m in range(MHP):
                        for kc in range(KHP_C):
                            nc.tensor.matmul(
                                ghp_ps[:, m:m + 1],
                                whpc[:, (m * KHP_C + kc) * 128:(m * KHP_C + kc + 1) * 128],
                                allB_c[:, kc:kc + 1],
                                start=False, stop=False,
                            )
                for m in range(MHP):
                    for j in range(KHP_I):
                        rhs = a_bf[:, j:j + 1] if j < 2 else ahp_bf[:, j - 2:j - 1]
                        nc.tensor.matmul(
                            ghp_ps[:, m:m + 1],
                            whpi[:, (m * KHP_I + j) * 128:(m * KHP_I + j + 1) * 128],
                            rhs, start=False, stop=(j == KHP_I - 1),
                        )
                hs_ps = psM.tile([128, 2], F32, tag="psM")
                for m in range(2):
                    for kc in range(KSUM):
                        nc.tensor.matmul(
                            hs_ps[:, m:m + 1],
                            wsum[:, (m * KSUM + kc) * 128:(m * KSUM + kc + 1) * 128],
                            allB_hl[:, hcol(kc):hcol(kc) + 1],
                            start=(kc == 0), stop=(kc == KSUM - 1),
                        )
                # h_sum_hat = 2*relu(W_sum@h + b_sum) = relu(2*psum + 2*b_sum)
                nc.scalar.activation(ph2_cur[:, 2:3], hs_ps[:, 0:1], AF.Relu,
                                     bias=bsum2[:, 0:1], scale=2.0)
                nc.scalar.activation(ph2_cur[:, 3:4], hs_ps[:, 1:2], AF.Relu,
                                     bias=bsum2[:, 1:2], scale=2.0)

                # --- B-side gate math (DVE/ACT); gates read PSUM directly ---
                acts_h = spool.tile([128, MHP], F32, tag="acts_h")
                # gate layout [i(0:4) f(4:8) o(8:12) g(12:16)]
                nc.scalar.activation(acts_h[:, 0:12], ghp_ps[:, 0:12], AF.Tanh, scale=0.5)
                nc.scalar.activation(acts_h[:, 12:16], ghp_ps[:, 12:16], AF.Tanh)
                hm1 = spool.tile([128, 4], F32, tag="hm1")
                hs1 = spool.tile([128, 4], F32, tag="hs1")
                nc.vector.tensor_mul(hm1[:], acts_h[:, 0:4], acts_h[:, 12:16])
                nc.vector.tensor_add(hs1[:], acts_h[:, 12:16], hm1[:])
                hm2 = spool.tile([128, 4], F32, tag="hm2")
                hs2 = spool.tile([128, 4], F32, tag="hs2")
                nc.vector.tensor_mul(hm2[:], acts_h[:, 4:8], ph2_cur[:])
                nc.vector.tensor_add(hs2[:], ph2_cur[:], hm2[:])
                hs2h = spool.tile([128, 4], F32, tag="hs2h")
                nc.vector.tensor_scalar_mul(hs2h[:], hs2[:], 0.5)
                payB = spool.tile([128, CB], BF, tag="payB")
                c2f = spool.tile([128, 4], F32, tag="c2f")
                nc.vector.tensor_add(c2f[:], hs1[:], hs2h[:])         # c_hat f32
                nc.vector.tensor_scalar_mul(payB[:, 0:4], c2f[:], 1.0 / WSC)
                tch = spool.tile([128, 4], F32, tag="tch")
                nc.scalar.activation(tch[:], c2f[:], AF.Tanh, scale=0.5)
                hm3 = spool.tile([128, 4], F32, tag="hm3")
                nc.vector.tensor_mul(hm3[:], acts_h[:, 8:12], tch[:])
                nc.vector.tensor_add(payB[:, 4:8], tch[:], hm3[:])    # h_hat (bf16)

                lhp_ps = psM.tile([128, 2], F32, tag="psM")
                for m in range(2):
                    for kc in range(4):
                        nc.tensor.matmul(
                            lhp_ps[:, m:m + 1],
                            wohp[:, (m * 4 + kc) * 128:(m * 4 + kc + 1) * 128],
                            payB[:, 4 + kc:5 + kc], start=(kc == 0), stop=(kc == 3),
                        )
                nc.vector.tensor_add(payB[:, 8:10], lhp_ps[:], bohp8[:])

                gB = gather(payB, CB, "ccB", "gB")
                tc.no_sync_barrier()

                if t + 1 < DEPTH:
                    ph2_next, payA = A_step(a_bf, allA_cur)
                    gA = gather(payA, CA, "ccA", "gA")

                # c-part readback split by rank halves: the W_hh@c chunks for
                # ranks 0-3 only wait on the first half
                allB_c = xpool.tile([128, NC * 4], BF, tag="allBc")
                nc.sync.dma_start(allB_c[:, 0:16], gB[0:4, :, 0:4].rearrange("r p c -> p r c"))
                nc.sync.dma_start(allB_c[:, 16:32], gB[4:NC, :, 0:4].rearrange("r p c -> p r c"))
                allB_hl = xpool.tile([128, NC * CHL], BF, tag="allBhl")
                nc.sync.dma_start(allB_hl[:], gB[:, :, 4:CB].rearrange("r p c -> p r c"))
                if t + 1 < DEPTH:
                    allA_cur = xpool.tile([128, NC * CA], BF, tag="allA")
                    nc.sync.dma_start(allA_cur[:], gA[:].rearrange("r p c -> p r c"))
                    ph2_cur = ph2_next

            # tail: last hp softmax
            softmax(allB_hl, 4, DEPTH - 1, outHP, "ahp")

            # transpose [128, 48] -> [48, 128] on PE so the output DMA writes
            # contiguous 512B rows instead of a 4-byte-packet scatter
            tr_ps = psM.tile([4 * DEPTH, 128], F32, tag="tr_ps", bufs=1)
            nc.tensor.transpose(tr_ps[:], outAB[:], ident[:])
            trf = spool.tile([4 * DEPTH, 128], F32, tag="trf", bufs=1)
            nc.vector.tensor_copy(trf[:], tr_ps[:])
            nc.sync.dma_start(
                out_e[:].rearrange("s t (m p) -> (s t m) p", p=128),
                trf[:],
            )
    nc.finalize()
    return nc


_NC_CACHE = None


def _get_nc():
    global _NC_CACHE
    if _NC_CACHE is None:
        _NC_CACHE = _build_nc()
    return _NC_CACHE


def _lhsT_pack(w_cat, n_m, n_k):
    """w_cat [n_m*128 rows, n_k*128 cols] -> SBUF image [128, n_m*n_k*128] where
    cols [(m*n_k+kc)*128 + j] on partition p = w_cat[m*128 + j, kc*128 + p]."""
    a = w_cat.reshape(n_m, 128, n_k, 128)           # [m, j, kc, p]
    return np.ascontiguousarray(a.transpose(3, 0, 2, 1).reshape(128, n_m * n_k * 128))


GATE_PERM = (0, 1, 3, 2)  # pytorch [i,f,g,o] -> kernel [i,f,o,g]


def _prep_in_maps(x_thought_vec_arch, x_thought_vec_arch_hp,
                  W_ih_a, W_hh_a, b_ih_a, b_hh_a, W_out_a, b_out_a,
                  W_sum, b_sum, W_ih_hp, W_hh_hp, b_ih_hp, b_hh_hp,
                  W_out_hp, b_out_hp):
    f32 = np.float32
    bf16 = ml_dtypes.bfloat16
    php = np.concatenate([
        np.concatenate([np.arange(SA * k, SA * (k + 1)),
                        HA + np.arange(SS * k, SS * (k + 1))])
        for k in range(NC)
    ])
    ba_full = (np.asarray(b_ih_a) + np.asarray(b_hh_a)).astype(f32)
    bhp_full = (np.asarray(b_ih_hp) + np.asarray(b_hh_hp)).astype(f32)
    ha0 = np.asarray(x_thought_vec_arch, f32).reshape(HA)
    hhp0 = np.asarray(x_thought_vec_arch_hp, f32).reshape(HHP)
    W_ih_a = np.asarray(W_ih_a, f32); W_hh_a = np.asarray(W_hh_a, f32)
    W_out_a = np.asarray(W_out_a, f32); W_sum = np.asarray(W_sum, f32)
    W_ih_hp = np.asarray(W_ih_hp, f32); W_hh_hp = np.asarray(W_hh_hp, f32)
    W_out_hp = np.asarray(W_out_hp, f32)
    b_out_a = np.asarray(b_out_a, f32); b_out_hp = np.asarray(b_out_hp, f32)
    b_sum = np.asarray(b_sum, f32)

    # init images in "2x" space, laid out like the gathered tiles
    initA = np.zeros((128, NC * CA), f32)
    initB = np.zeros((128, NC * CHL), f32)
    ha0_2 = 2.0 * ha0
    hhp0_2 = (2.0 * hhp0)[php]
    for r in range(NC):
        for j in range(2):
            initA[:, r * CA + j] = ha0_2[r * SA + j * 128: r * SA + (j + 1) * 128]
        for q in range(4):
            initB[:, r * CHL + q] = hhp0_2[r * SHP + q * 128: r * SHP + (q + 1) * 128]

    in_maps = []
    for k in range(NC):
        ja = np.arange(SA * k, SA * (k + 1))
        rows_a = np.concatenate([g * HA + ja for g in GATE_PERM])
        wa_cat = np.concatenate([W_ih_a[rows_a], 0.5 * W_hh_a[rows_a]], axis=1)
        jhp = php[SHP * k: SHP * (k + 1)]
        rows_hp = np.concatenate([g * HHP + jhp for g in GATE_PERM])
        whpc_cat = (0.5 * WSC) * W_hh_hp[rows_hp][:, php]
        whpi_cat = W_ih_hp[rows_hp]
        js = np.arange(SS * k, SS * (k + 1))
        wsum_p = 0.5 * W_sum[js][:, php]
        woa_p = 0.5 * W_out_a[:, ja]
        wohp_p = 0.5 * W_out_hp[:, jhp]
        in_maps.append({
            "wa": _lhsT_pack(wa_cat, MA, KA).astype(bf16),
            "wsum": _lhsT_pack(wsum_p, 2, KSUM).astype(bf16),
            "whpc": _lhsT_pack(whpc_cat, MHP, KHP_C).astype(ml_dtypes.float8_e4m3),
            "whpi": _lhsT_pack(whpi_cat, MHP, KHP_I).astype(bf16),
            "woa": _lhsT_pack(woa_p, 2, 2).astype(bf16),
            "wohp": _lhsT_pack(wohp_p, 2, 4).astype(bf16),
            "baT": np.ascontiguousarray(ba_full[rows_a].reshape(MA, 128)).astype(bf16),
            "bsum2": np.ascontiguousarray((2.0 * b_sum[js]).reshape(2, 128).T),
            "bhpT": np.ascontiguousarray(bhp_full[rows_hp].reshape(MHP, 128)).astype(bf16),
            "identb": np.eye(MHP, dtype=f32).astype(bf16),
            "boa8": np.ascontiguousarray((b_out_a / NC).reshape(2, 128).T),
            "bohp8": np.ascontiguousarray((b_out_hp / NC).reshape(2, 128).T),
            "initA": initA.astype(bf16),
            "initB": initB.astype(bf16),
            "ident": np.eye(128, dtype=f32),
        })
    return in_maps


def _run(in_maps, trace=False):
    nc = _get_nc()
    return run_bass_kernel_spmd(nc, in_maps, core_ids=list(range(NC)), trace=trace)


def kernel(**inputs):
    in_maps = _prep_in_maps(**{k: np.asarray(v) for k, v in inputs.items()})
    res = _run(in_maps, trace=False)
    out = np.asarray(res.results[0]["out"], np.float32)
    return out[0][None], out[1][None]


def kernel_traced(**inputs):
    """Like kernel() but with NTFF profiling; returns ((arch, arch_hp), exec_time_ns)."""
    try:
        import ntff_hook
        ntff_hook.install()
    except Exception:
        pass
    in_maps = _prep_in_maps(**{k: np.asarray(v) for k, v in inputs.items()})
    res = _run(in_maps, trace=True)
    out = np.asarray(res.results[0]["out"], np.float32)
    return (out[0][None], out[1][None]), res.exec_time_ns
